# revision 22
# baseline (speedup 1.0000x reference)
"""Trainium2 Bass kernel for EM matrix-capsule routing (nn_MatrixRouting).

Problem shapes (hardcoded): votes [4, 1152, 1152, 17] f32, beta_v [1,32,1,1],
beta_a [1,32,1], output_dim=32, num_routing=3. Output [4, 32, 6, 6, 17].

Strategy: shard the output-capsule axis C=1152 across 8 cores (144 each).
Host pre-transposes each core's vote shard to a p-major (p,c8)-on-partition
fp16 layout: 18 tiles of [128 = 16 pose x 8 caps, I=1152] per batch; the
shard stays SBUF-resident across all 3 EM iterations.

Per-partition EM params (mu, -g, lnE) make the Gaussian 1-3 ops; p-sums and
row-sums are tiny shared-selector matmuls on TensorE; the q -> (c,p) row
replication is a single stride-0-partition SBUF->SBUF DMA per tile (p-major
makes the replicated view contiguous); stats products run as 2x-mode
tensor_tensor with 4x-mode tensor_scalar accumulates (a few tiles on
GpSimd for balance). One activation-table set (exp/ln/square/identity)
serves the whole kernel: sqrt -> 0.5*ln, sigmoid -> exp + tiny reciprocal.
The only cross-core exchange is the [1, I] R-normalizer row, AllReduce'd
per (batch, iteration) and hidden under the paired batch's compute.
"""

import math
import numpy as np
from contextlib import ExitStack

# ---- problem constants (hardcoded per the task contract) ----
B = 4
I_DIM = 1152
C_DIM = 1152
P_DIM = 16
D_DIM = 17
N_CORES = 8
NUM_ROUTING = 3
O_DIM = 32
WW = 36  # w*w = 6*6 positions per output capsule

CL = C_DIM // N_CORES        # 144 local caps
NT = CL * P_DIM // 128       # 18 tiles of [128, I]
NTM = 16                     # tiles whose caps fit the main [128, I] c-pack
NOV = CL - 128               # 16 overflow caps (tiles 16, 17)
NS = 3                       # i-slices per tile for PSUM bank alignment
SW = I_DIM // NS             # 384 columns per slice

EPS = 1e-10
LAMBDA = 1e-4
LN_2PI = math.log(2.0 * math.pi)

# ---- tuning knobs ----
N_ACT = 18                   # tiles using the scalar-engine Square form of p1
GP_TILES = (5, 11)           # p2 tiles whose products run on GpSimd

_NC_CACHE = {}


def _patch_tile_drain():
    """This walrus build only accepts one sync-wait on a CTRL instruction;
    spread the Tile exit-drain waits across single-wait NOPs."""
    import concourse.tile as tile
    import concourse.mybir as mybir
    from concourse.vector_clock import ScopedClock

    if getattr(tile.TileContext, "_drain_patched", False):
        return

    def _patched(self, tick_clock, wait_clock):
        nc = self.nc
        probe = nc.sync.nop()
        wait_clock.add_sem_waits(
            probe.ins, ScopedClock({None: tick_clock.global_clock})
        )
        waits = list(probe.ins.sync_info.on_wait) if probe.ins.sync_info else []
        if probe.ins.sync_info:
            probe.ins.sync_info.on_wait = waits[:1]
        for w in waits[1:]:
            n2 = nc.sync.nop()
            if n2.ins.sync_info is None:
                n2.ins.sync_info = mybir.SyncInfo(on_wait=[w], on_update=[])
            else:
                n2.ins.sync_info.on_wait = [w]
        nc.sync.drain()
        nc.all_engine_barrier()
        assert self.sems is not None
        popped = nc._tile_sem_poison_stack.pop()
        assert popped is self._sem_poison
        nc.clear_and_free_semaphores(list(self.sems.allocated().values()))
        nc.all_engine_barrier()

    tile.TileContext._drain_and_barrier = _patched
    tile.TileContext._drain_patched = True


def _split_sync_waits(nc, max_waits=1):
    """This walrus build accepts at most one sync-wait per instruction;
    move excess waits onto preceding same-engine NOPs."""
    import concourse.mybir as mybir

    uid = [0]
    for fn in nc.m.functions:
        for bb in fn.blocks:
            insts = bb.instructions
            out = []
            for inst in insts:
                si = inst.sync_info
                if si is not None and si.on_wait and len(si.on_wait) > max_waits:
                    waits = list(si.on_wait)
                    keep = waits[-max_waits:]
                    for w in waits[:-max_waits]:
                        uid[0] += 1
                        nop = mybir.InstNoOp(
                            name=f"I-waitsplit-{uid[0]}", ins=[], outs=[])
                        nop.engine = inst.engine
                        nop.sync_info = mybir.SyncInfo(on_wait=[w], on_update=[])
                        out.append(nop)
                    si.on_wait = keep
                out.append(inst)
            bb.instructions = out
    return nc


def build_nc(num_routing=NUM_ROUTING, split_waits=True):
    """Build the per-core SPMD Bass program (identical on every core)."""
    import concourse.bass as bass
    import concourse.mybir as mybir
    import concourse.tile as tile

    _patch_tile_drain()

    f32 = mybir.dt.float32
    f16 = mybir.dt.float16
    ALU = mybir.AluOpType
    ACTF = mybir.ActivationFunctionType

    nc = bass.Bass()
    vt_in = nc.declare_dram_parameter("vt", [B, NT, 128, I_DIM], f16, isOutput=False)
    at_in = nc.declare_dram_parameter("at", [B, 128, I_DIM], f16, isOutput=False)
    av_in = nc.declare_dram_parameter("av", [B, NOV, I_DIM], f16, isOutput=False)
    bdp_in = nc.declare_dram_parameter("bdp", [128, NTM * 128], f16, isOutput=False)
    bdov_in = nc.declare_dram_parameter("bdov", [128, 2 * NOV], f16, isOutput=False)
    am_in = nc.declare_dram_parameter("am", [128, 128], f16, isOutput=False)
    mm_in = nc.declare_dram_parameter("mm", [128, NT], f16, isOutput=False)
    m2_in = nc.declare_dram_parameter("m2", [NOV, 2], f16, isOutput=False)
    bd8_in = nc.declare_dram_parameter("bd8", [128, 8], f16, isOutput=False)
    bdt8_in = nc.declare_dram_parameter("bdt8", [8, 128], f16, isOutput=False)
    bv8_in = nc.declare_dram_parameter("bv8", [8, NT], f32, isOutput=False)
    ba8_in = nc.declare_dram_parameter("ba8", [8, NT], f32, isOutput=False)
    out_mu = nc.declare_dram_parameter("out_mu", [B, NT, 128], f32, isOutput=True)
    out_a = nc.declare_dram_parameter("out_a", [B, NT, 8], f32, isOutput=True)
    rs_loc = nc.dram_tensor("rs_loc", [B, I_DIM], f32)
    rs_sh = nc.dram_tensor("rs_sh", [B, I_DIM], f32, addr_space="Shared")
    qd = nc.dram_tensor("qd", [B, CL, I_DIM], f16)
    rcpd = nc.dram_tensor("rcpd", [B, 1, I_DIM], f16)

    groups = [list(range(N_CORES))]

    with tile.TileContext(nc) as tc, ExitStack() as ctx:
        pconst = ctx.enter_context(tc.tile_pool(name="const", bufs=1))
        pv = ctx.enter_context(tc.tile_pool(name="vt", bufs=2))
        pat = ctx.enter_context(tc.tile_pool(name="at", bufs=3))
        pwk = ctx.enter_context(tc.tile_pool(name="wk", bufs=2))
        pe_ = ctx.enter_context(tc.tile_pool(name="ex", bufs=2))
        pap = ctx.enter_context(tc.tile_pool(name="apS", bufs=2))
        pqc = ctx.enter_context(tc.tile_pool(name="qc", bufs=3))
        ps1 = ctx.enter_context(tc.tile_pool(name="s1o", bufs=2))
        prs = ctx.enter_context(tc.tile_pool(name="rs", bufs=2))
        psm = ctx.enter_context(tc.tile_pool(name="sm", bufs=2))
        # PSUM budget (8 banks): ap ring 3 (shared with rp) + ov 3 + q ring 2
        pps_ap = ctx.enter_context(tc.tile_pool(name="pap", bufs=1, space="PSUM"))
        pps_ov = ctx.enter_context(tc.tile_pool(name="pov", bufs=1, space="PSUM"))
        pps_q = ctx.enter_context(tc.tile_pool(name="pq", bufs=2, space="PSUM"))

        # ---- constants ----
        bdp = pconst.tile([128, NTM, 128], f16)
        nc.sync.dma_start(bdp[:].rearrange("p a b -> p (a b)"), bdp_in[:])
        bdov = pconst.tile([128, 2, NOV], f16)
        nc.sync.dma_start(bdov[:].rearrange("p a b -> p (a b)"), bdov_in[:])
        amat = pconst.tile([128, 128], f16)
        nc.sync.dma_start(amat[:], am_in[:])
        mmask = pconst.tile([128, NT], f16)
        nc.sync.dma_start(mmask[:], mm_in[:])
        m2mask = pconst.tile([NOV, 2], f16)
        nc.sync.dma_start(m2mask[:], m2_in[:])
        bd8 = pconst.tile([128, 8], f16)
        nc.sync.dma_start(bd8[:], bd8_in[:])
        bdt8 = pconst.tile([8, 128], f16)
        nc.sync.dma_start(bdt8[:], bdt8_in[:])
        bv8 = pconst.tile([8, NT], f32)
        nc.sync.dma_start(bv8[:], bv8_in[:])
        ba8 = pconst.tile([8, NT], f32)
        nc.sync.dma_start(ba8[:], ba8_in[:])
        onesA = pconst.tile([128, 1], f16)
        nc.vector.memset(onesA[:], 1.0)
        onesV = pconst.tile([NOV, 1], f16)
        nc.vector.memset(onesV[:], 1.0)
        eps_col = pconst.tile([128, 1], f32)
        nc.vector.memset(eps_col[:], EPS)

        vts, ats, avs = {}, {}, {}
        loaded = set()

        def load(b):
            if b >= B or b in loaded:
                return
            loaded.add(b)
            vt = pv.tile([128, NT, I_DIM], f16, tag="vt", name=f"vt{b}")
            nc.sync.dma_start(
                vt[:], vt_in[b].rearrange("t p i -> p t i"))
            at = pat.tile([128, I_DIM], f16, tag="at", name=f"at{b}")
            nc.sync.dma_start(at[:], at_in[b])
            av = pat.tile([NOV, I_DIM], f16, tag="av", name=f"av{b}")
            nc.sync.dma_start(av[:], av_in[b])
            vts[b], ats[b], avs[b] = vt, at, av

        params = {}   # b -> (mu, negg, lnE, sqg, bng) [128, NT] f32 tiles

        def p1_gen(b, k):
            """Gaussian weights e -> per-cap p-sum ap (f16 SBUF), local
            row-sum, AllReduce launch. Yields once per tile."""
            vt = vts[b]
            mu_t, negg_t, lnE_t, sqg_t, bng_t = params[b]
            ap_ps = pps_ap.tile([128, NS, 512], f32, tag="ap", name=f"ap{b}_{k}")
            ov_ps = pps_ov.tile([NOV, NS, 512], f32, tag="ov", name=f"ov{b}_{k}")
            for t in range(NT):
                V = vt[:, t, :]
                e = pe_.tile([128, I_DIM], f16, tag="e", name=f"e{b}{k}{t}")
                if t < N_ACT:
                    # ACT form: u = (sqrt(g) V - sqrt(g) mu)^2 ; e = exp(-u+lnE)
                    u = pwk.tile([128, I_DIM], f16, tag="d", name=f"u{b}{k}{t}")
                    nc.scalar.activation(u[:], V, ACTF.Square,
                                         bias=bng_t[:, t : t + 1],
                                         scale=sqg_t[:, t : t + 1])
                    nc.scalar.activation(e[:], u[:], ACTF.Exp,
                                         bias=lnE_t[:, t : t + 1], scale=-1.0)
                else:
                    # DVE form: d2 = (V - mu)^2 ; e = exp(negg d2 + lnE)
                    d = pwk.tile([128, I_DIM], f16, tag="d", name=f"d{b}{k}{t}")
                    nc.vector.tensor_scalar(
                        d[:], V, mu_t[:, t : t + 1], None, op0=ALU.subtract)
                    d2 = pwk.tile([128, I_DIM], f16, tag="z", name=f"d2{b}{k}{t}")
                    nc.vector.tensor_tensor(d2[:], d[:], d[:], op=ALU.mult)
                    nc.scalar.activation(e[:], d2[:], ACTF.Exp,
                                         bias=lnE_t[:, t : t + 1],
                                         scale=negg_t[:, t : t + 1])
                # p-sum: rows (p, c8) of tile t -> cap row 8t + c8
                for s in range(NS):
                    rhs = e[:, s * SW : (s + 1) * SW]
                    if t < NTM:
                        nc.tensor.matmul(ap_ps[:, s, 0:SW], bdp[:, t, :], rhs,
                                         start=(t == 0), stop=(t == NTM - 1))
                    else:
                        nc.tensor.matmul(ov_ps[:, s, 0:SW], bdov[:, t - NTM, :],
                                         rhs, start=(t == NTM), stop=(t == NT - 1))
                yield
            # free PSUM early: f16 copies of the cap-packed ap
            apS = pap.tile([128, I_DIM], f16, tag="apS", name=f"apS{b}{k}")
            nc.scalar.activation(
                apS[:].rearrange("p (s x) -> p s x", s=NS),
                ap_ps[:, :, 0:SW], ACTF.Identity)
            ovS = pap.tile([NOV, I_DIM], f16, tag="ovS", name=f"ovS{b}{k}")
            nc.scalar.activation(
                ovS[:].rearrange("p (s x) -> p s x", s=NS),
                ov_ps[:, :, 0:SW], ACTF.Identity)
            # local row-sum over all c: ones-matmuls on the f16 ap copies
            # (reuses the ap psum banks, which are dead after the copies)
            rp = pps_ap.tile([128, NS, 512], f32, tag="ap", name=f"rp{b}{k}")
            for s in range(NS):
                nc.tensor.matmul(rp[0:1, s, 0:SW], onesA[:],
                                 apS[:, s * SW : (s + 1) * SW],
                                 start=True, stop=False)
                nc.tensor.matmul(rp[0:1, s, 0:SW], onesV[:],
                                 ovS[:, s * SW : (s + 1) * SW],
                                 start=False, stop=True)
            rs_row = prs.tile([1, I_DIM], f32, tag="rsrow", name=f"rsrow{b}{k}")
            nc.scalar.activation(rs_row[:].rearrange("p (s x) -> p s x", s=NS),
                                 rp[0:1, :, 0:SW], ACTF.Identity)
            nc.sync.dma_start(rs_loc[b], rs_row[:])
            nc.gpsimd.collective_compute(
                "AllReduce", ALU.add, replica_groups=groups,
                ins=[rs_loc[b]], outs=[rs_sh[b]])
            return apS, ovS

        def qmini(b, k, apS, ovS):
            """R-normalize + vote-activation weight -> cap-packed q in SBUF."""
            # read the AllReduce'd row as [128, 9] so the iterative
            # reciprocal runs partition-parallel (~60 cyc, not ~7.5us)
            rsg = prs.tile([128, 9], f32, tag="rsg", name=f"rsg{b}{k}")
            nc.sync.dma_start(rsg[:], rs_sh[b].rearrange("(r j) -> r j", j=9))
            nc.vector.tensor_scalar(rsg[:], rsg[:], EPS, None, op0=ALU.add)
            rcpf = prs.tile([128, 9], f32, tag="rcpf", name=f"rcpf{b}{k}")
            nc.vector.reciprocal(rcpf[:], rsg[:])
            rcp1 = prs.tile([128, 9], f16, tag="rcp1", name=f"rcp1{b}{k}")
            with nc.allow_low_precision(
                    reason="f16 R-normalizer; gate tolerance 2e-2"):
                nc.vector.tensor_scalar(rcp1[:], rcpf[:], 1.0, None,
                                        op0=ALU.mult)
            # broadcast the row to all partitions via a DRAM bounce
            nc.sync.dma_start(
                rcpd[b].rearrange("o (r j) -> (o r) j", j=9), rcp1[:])
            rcp = prs.tile([128, I_DIM], f16, tag="rcp", name=f"rcp{b}{k}")
            nc.sync.dma_start(rcp[:], rcpd[b].broadcast_to((128, I_DIM)))
            qp = ps1.tile([128, I_DIM], f16, tag="qp", name=f"qp{b}{k}")
            nc.vector.tensor_tensor(qp[:], apS[:], rcp[:], op=ALU.mult)
            nc.vector.tensor_tensor(qp[:], qp[:], ats[b][:], op=ALU.mult)
            qv = ps1.tile([NOV, I_DIM], f16, tag="qv", name=f"qv{b}{k}")
            nc.vector.tensor_tensor(qv[:], ovS[:], rcp[0:NOV, :], op=ALU.mult)
            nc.vector.tensor_tensor(qv[:], qv[:], avs[b][:], op=ALU.mult)
            nc.sync.dma_start(qd[b, 0:128], qp[:])
            nc.sync.dma_start(qd[b, 128:CL], qv[:])
            return qp, qv

        def p2_gen(b, k, qp, qv):
            """Stats via DMA-broadcast q + 2x products + 4x accumulates,
            then the small per-cap math."""
            vt = vts[b]
            S1 = psm.tile([128, NT], f32, tag="S1", name=f"S1{b}{k}")
            S2 = psm.tile([128, NT], f32, tag="S2", name=f"S2{b}{k}")
            # S0 (= sum_i q) per cap via in-place identity + accum, then
            # selector-matmuls spread it to (p,c8) rows and [8, NT] form
            S0q = psm.tile([128, 1], f32, tag="S0q", name=f"S0q{b}{k}")
            nc.vector.tensor_scalar(qp[:], qp[:], 1.0, 0.0, op0=ALU.mult,
                                    op1=ALU.add, accum_out=S0q[:])
            S0v = psm.tile([NOV, 1], f32, tag="S0v", name=f"S0v{b}{k}")
            nc.vector.tensor_scalar(qv[:], qv[:], 1.0, 0.0, op0=ALU.mult,
                                    op1=ALU.add, accum_out=S0v[:])
            Bm = psm.tile([128, NT], f16, tag="Bm", name=f"Bm{b}{k}")
            nc.vector.tensor_scalar(Bm[:], mmask[:], S0q[:], None, op0=ALU.mult)
            B2 = psm.tile([NOV, 2], f16, tag="B2", name=f"B2{b}{k}")
            nc.vector.tensor_scalar(B2[:], m2mask[:], S0v[:], None, op0=ALU.mult)
            s0w_ps = pps_q.tile([128, 512], f32, tag="q", name=f"s0w{b}{k}")
            nc.tensor.matmul(s0w_ps[:, 0:NTM], amat[:], Bm[:, 0:NTM],
                             start=True, stop=True)
            nc.tensor.matmul(s0w_ps[:, NTM:NT], amat[0:NOV, :], B2[:],
                             start=True, stop=True)
            # S0 arranged [8, NT] for the per-cap cost math
            s08_ps = pps_q.tile([128, 512], f32, tag="q", name=f"s08{b}{k}")
            nc.tensor.matmul(s08_ps[0:8, 0:NTM], amat[:, 0:8], Bm[:, 0:NTM],
                             start=True, stop=True)
            nc.tensor.matmul(s08_ps[0:8, NTM:NT], amat[0:NOV, 0:8], B2[:],
                             start=True, stop=True)

            for t in range(NT):
                V = vt[:, t, :]
                qcp = pqc.tile([128, I_DIM], f16, tag="qc", name=f"qc{b}{k}{t}")
                if k == 0:
                    if t < NTM:
                        src = at_in[b, 8 * t : 8 * t + 8, :]
                    else:
                        src = av_in[b, 8 * (t - NTM) : 8 * (t - NTM) + 8, :]
                else:
                    if t < NTM:
                        src = qd[b, 8 * t : 8 * t + 8, :]
                    else:
                        src = qd[b, 128 + 8 * (t - NTM) :
                                 128 + 8 * (t - NTM) + 8, :]
                nc.sync.dma_start(qcp[:], src.partition_broadcast(16))
                s1o = ps1.tile([128, I_DIM], f16, tag="s1o", name=f"s1o{b}{k}{t}")
                s2o = ps1.tile([128, I_DIM], f16, tag="s2o", name=f"s2o{b}{k}{t}")
                if t in GP_TILES:
                    nc.gpsimd.tensor_tensor(s1o[:], qcp[:], V, op=ALU.mult)
                else:
                    nc.vector.tensor_tensor(s1o[:], qcp[:], V, op=ALU.mult)
                nc.vector.tensor_scalar(s1o[:], s1o[:], 1.0, 0.0, op0=ALU.mult,
                                        op1=ALU.add,
                                        accum_out=S1[:, t : t + 1])
                if t in GP_TILES:
                    nc.gpsimd.tensor_tensor(s2o[:], s1o[:], V, op=ALU.mult)
                else:
                    nc.vector.tensor_tensor(s2o[:], s1o[:], V, op=ALU.mult)
                nc.vector.tensor_scalar(s2o[:], s2o[:], 1.0, 0.0, op0=ALU.mult,
                                        op1=ALU.add,
                                        accum_out=S2[:, t : t + 1])
                yield

            # ---- small math on [128, NT] f32 (p-major rows) ----
            rS = psm.tile([128, NT], f32, tag="rS", name=f"rS{b}{k}")
            nc.vector.reciprocal(rS[:], s0w_ps[:, 0:NT])
            mu = psm.tile([128, NT], f32, tag="mu", name=f"mu{b}{k}")
            nc.vector.tensor_tensor(mu[:], S1[:], rS[:], op=ALU.mult)
            ex2 = psm.tile([128, NT], f32, tag="ex2", name=f"ex2{b}{k}")
            nc.vector.tensor_tensor(ex2[:], S2[:], rS[:], op=ALU.mult)
            mu2 = psm.tile([128, NT], f32, tag="mu2", name=f"mu2{b}{k}")
            nc.vector.tensor_tensor(mu2[:], mu[:], mu[:], op=ALU.mult)
            sig2 = psm.tile([128, NT], f32, tag="sig2", name=f"sig2{b}{k}")
            nc.vector.tensor_tensor(sig2[:], ex2[:], mu2[:], op=ALU.subtract)
            nc.vector.tensor_scalar_max(sig2[:], sig2[:], 1e-12)
            # log sigma = 0.5 ln(sig2); the 0.5 is folded into bd8/lnE uses
            L = psm.tile([128, NT], f16, tag="L", name=f"L{b}{k}")
            nc.scalar.activation(L[:], sig2[:], ACTF.Ln)
            # per-cap cost: smp[c8, t] = sum_p L ; bd8 entries are 0.5
            smp = pps_q.tile([128, 512], f32, tag="q", name=f"smp{b}{k}")
            nc.tensor.matmul(smp[0:8, 0:NT], bd8[:], L[:], start=True, stop=True)
            c1 = psm.tile([8, NT], f32, tag="c1", name=f"c1{b}{k}")
            nc.vector.tensor_tensor(c1[:], smp[0:8, 0:NT], bv8[:], op=ALU.add)
            c2 = psm.tile([8, NT], f32, tag="c2", name=f"c2{b}{k}")
            nc.vector.tensor_tensor(c2[:], c1[:], s08_ps[0:8, 0:NT], op=ALU.mult)
            wk = (1.0 / O_DIM) if k == 0 else 1.0
            ain = psm.tile([8, NT], f32, tag="ain", name=f"ain{b}{k}")
            nc.vector.scalar_tensor_tensor(
                ain[:], c2[:], -wk, ba8[:], op0=ALU.mult, op1=ALU.add)
            # a = sigmoid(LAMBDA * ain) = 1 / (1 + exp(-LAMBDA * ain))
            ea = psm.tile([8, NT], f32, tag="ea", name=f"ea{b}{k}")
            nc.scalar.activation(ea[:], ain[:], ACTF.Exp, scale=-LAMBDA)
            ua = psm.tile([8, NT], f32, tag="ua", name=f"ua{b}{k}")
            nc.vector.tensor_scalar(ua[:], ea[:], 1.0, None, op0=ALU.add)
            a8 = psm.tile([8, NT], f32, tag="a8", name=f"a8{b}{k}")
            nc.vector.reciprocal(a8[:], ua[:])

            if k == num_routing - 1:
                nc.sync.dma_start(out_mu[b].rearrange("t r -> r t"), mu[:])
                nc.sync.dma_start(out_a[b].rearrange("t c -> c t"), a8[:])
            else:
                a816 = psm.tile([8, NT], f16, tag="a816", name=f"a816{b}{k}")
                with nc.allow_low_precision(reason="a broadcast; tol 2e-2"):
                    nc.vector.tensor_scalar(a816[:], a8[:], 1.0, None,
                                            op0=ALU.mult)
                arep = pps_q.tile([128, 512], f32, tag="q", name=f"ar{b}{k}")
                nc.tensor.matmul(arep[:, 0:NT], bdt8[:], a816[:],
                                 start=True, stop=True)
                lnA = psm.tile([128, NT], f32, tag="lnA", name=f"lnA{b}{k}")
                nc.scalar.activation(lnA[:], arep[:, 0:NT], ACTF.Ln, bias=eps_col[:])
                # lnE = lnA - 0.5 ln(sig2) - 0.5 ln(2pi)
                lnE = psm.tile([128, NT], f32, tag="lnE", name=f"lnE{b}{k}")
                nc.vector.scalar_tensor_tensor(
                    lnE[:], L[:], -0.5, lnA[:], op0=ALU.mult, op1=ALU.add)
                nc.vector.tensor_scalar(lnE[:], lnE[:], -0.5 * LN_2PI, None,
                                        op0=ALU.add)
                rsig = psm.tile([128, NT], f32, tag="rsig", name=f"rv{b}{k}")
                nc.vector.reciprocal(rsig[:], sig2[:])
                negg = psm.tile([128, NT], f32, tag="negg", name=f"ng{b}{k}")
                nc.vector.tensor_scalar_mul(negg[:], rsig[:], -0.5)
                if N_ACT > 0:
                    # sqrt(g) = exp(0.5 ln(0.5 rsig)); bng = -sqrt(g) mu
                    lng = psm.tile([128, NT], f32, tag="lng", name=f"lg{b}{k}")
                    nc.scalar.activation(lng[:], rsig[:], ACTF.Ln, scale=0.5)
                    sqg = psm.tile([128, NT], f32, tag="sqg", name=f"sq{b}{k}")
                    nc.scalar.activation(sqg[:], lng[:], ACTF.Exp, scale=0.5)
                    bng = psm.tile([128, NT], f32, tag="bng", name=f"bg{b}{k}")
                    nc.vector.scalar_tensor_tensor(
                        bng[:], sqg[:], -1.0, mu[:], op0=ALU.mult, op1=ALU.mult)
                else:
                    sqg = bng = negg
                params[b] = (mu, negg, lnE, sqg, bng)

        # ---------------- schedule: batch pairs, AR hidden ----------------
        def drain(g):
            r = None
            while True:
                try:
                    r = next(g)
                except StopIteration as e:
                    return e.value

        for b0 in range(0, B, 2):
            b1 = b0 + 1
            load(b0)
            load(b1)
            load(b0 + 2)
            drain(p2_gen(b0, 0, ats[b0], avs[b0]))
            drain(p2_gen(b1, 0, ats[b1], avs[b1]))
            for k in range(1, num_routing):
                h0 = drain(p1_gen(b0, k))
                h1 = drain(p1_gen(b1, k))
                q0 = qmini(b0, k, *h0)
                drain(p2_gen(b0, k, *q0))
                q1 = qmini(b1, k, *h1)
                drain(p2_gen(b1, k, *q1))

    if split_waits:
        _split_sync_waits(nc)
    return nc


# ------------------------- host-side wrapper ----------------------------

def make_consts():
    """Selector/mask constants for the p-major (p, c8) packing."""
    # p-sum selectors: rows (8p + c8) of tile t -> cap col 8t + c8
    bdp = np.zeros((128, NTM, 128), np.float16)
    for t in range(NTM):
        for p in range(16):
            for c8 in range(8):
                bdp[8 * p + c8, t, 8 * t + c8] = 1.0
    bdov = np.zeros((128, 2, NOV), np.float16)
    for tv in range(2):
        for p in range(16):
            for c8 in range(8):
                bdov[8 * p + c8, tv, 8 * tv + c8] = 1.0
    # S0 spread: A[k, r] = 1 iff k % 8 == r % 8
    amat = np.zeros((128, 128), np.float16)
    for kk in range(128):
        for r in range(kk % 8, 128, 8):
            amat[kk, r] = 1.0
    # tile masks: M[k, t] = 1 iff k // 8 == t
    mmask = np.zeros((128, NT), np.float16)
    for kk in range(128):
        mmask[kk, kk // 8] = 1.0
    m2mask = np.zeros((NOV, 2), np.float16)
    for kk in range(NOV):
        m2mask[kk, kk // 8] = 1.0
    # p-reduce within cap, folded 0.5 for log sigma = 0.5 ln sig2
    bd8 = np.zeros((128, 8), np.float16)
    for p in range(16):
        for c8 in range(8):
            bd8[8 * p + c8, c8] = 0.5
    # a broadcast: [8, NT] -> (p, c8) rows
    bdt8 = np.zeros((8, 128), np.float16)
    for c8 in range(8):
        for p in range(16):
            bdt8[c8, 8 * p + c8] = 1.0
    return bdp, bdov, amat, mmask, m2mask, bd8, bdt8


def _get_nc():
    key = "full"
    if key not in _NC_CACHE:
        _NC_CACHE[key] = build_nc()
    return _NC_CACHE[key]


def make_in_maps(votes, beta_v, beta_a):
    """votes [B, I, C, D] f32 -> per-core input dicts (p-major packing)."""
    bvc = 16.0 * np.repeat(beta_v.reshape(-1), WW)   # [C], pre-scaled by P
    bac = np.repeat(beta_a.reshape(-1), WW)
    bdp_np, bdov_np, am_np, mm_np, m2_np, bd8_np, bdt8_np = make_consts()
    vt_all = np.ascontiguousarray(votes.transpose(0, 2, 3, 1))  # [B, C, D, I]
    in_maps = []
    for c in range(N_CORES):
        sl = slice(c * CL, (c + 1) * CL)
        blk = vt_all[:, sl]                               # [B, CL, D, I]
        pose = blk[:, :, :P_DIM, :].astype(np.float16)    # [B, CL, 16, I]
        # [B, CL=18*8 caps, 16 pose, I] -> [B, t, p, c8, i] -> [B,t,8p+c8,i]
        vt = np.ascontiguousarray(
            pose.reshape(B, NT, 8, P_DIM, I_DIM).transpose(0, 1, 3, 2, 4)
            .reshape(B, NT, 128, I_DIM))
        acts = blk[:, :, P_DIM, :].astype(np.float16)     # [B, CL, I]
        at = np.ascontiguousarray(acts[:, :128, :])
        av = np.ascontiguousarray(acts[:, 128:, :])
        # per-cap consts in [c8, t] layout: col t, row c8 -> cap 8t + c8
        cl_idx = np.arange(CL)
        bv8 = np.ascontiguousarray(
            bvc[c * CL + cl_idx].reshape(NT, 8).T.astype(np.float32))
        ba8 = np.ascontiguousarray(
            bac[c * CL + cl_idx].reshape(NT, 8).T.astype(np.float32))
        in_maps.append({
            "vt": vt, "at": at, "av": av,
            "bdp": np.ascontiguousarray(bdp_np.reshape(128, NTM * 128)),
            "bdov": np.ascontiguousarray(bdov_np.reshape(128, 2 * NOV)),
            "am": am_np, "mm": mm_np, "m2": m2_np,
            "bd8": bd8_np, "bdt8": bdt8_np,
            "bv8": bv8, "ba8": ba8,
        })
    return in_maps


def assemble_output(results):
    """Per-core out_mu [B, NT, 128] + out_a [B, NT, 8] -> [B, O, w, w, D].

    out_mu rows are p-major: row (8p + c8) of tile t = cap 8t+c8, pose p.
    """
    full = np.zeros((B, C_DIM, D_DIM), np.float32)
    for c in range(N_CORES):
        om = np.asarray(results[c]["out_mu"])             # [B, NT, 128]
        oa = np.asarray(results[c]["out_a"])              # [B, NT, 8]
        sl = slice(c * CL, (c + 1) * CL)
        # [B, t, (p, c8)] -> [B, t, c8, p] -> [B, CL, P]
        mu = om.reshape(B, NT, P_DIM, 8).transpose(0, 1, 3, 2)
        full[:, sl, :P_DIM] = mu.reshape(B, CL, P_DIM)
        full[:, sl, P_DIM] = oa.reshape(B, CL)
    w = int(math.sqrt(C_DIM // O_DIM))
    return full.reshape(B, O_DIM, w, w, D_DIM).astype(np.float32)


def kernel(**inputs) -> np.ndarray:
    from concourse.bass_utils import run_bass_kernel_spmd

    votes = np.ascontiguousarray(np.asarray(inputs["votes"], dtype=np.float32))
    beta_v = np.asarray(inputs["beta_v"], dtype=np.float32)
    beta_a = np.asarray(inputs["beta_a"], dtype=np.float32)
    output_dim = int(np.asarray(inputs["output_dim"]))
    num_routing = int(np.asarray(inputs["num_routing"]))
    assert votes.shape == (B, I_DIM, C_DIM, D_DIM), votes.shape
    assert output_dim == O_DIM and num_routing == NUM_ROUTING

    nc = _get_nc()
    in_maps = make_in_maps(votes, beta_v, beta_a)
    res = run_bass_kernel_spmd(nc, in_maps, list(range(N_CORES)))
    return assemble_output([res.results[i] for i in range(N_CORES)])


# revision 24
# speedup vs baseline: 1.0958x; 1.0958x over previous
"""Trainium2 Bass kernel for EM matrix-capsule routing (nn_MatrixRouting).

Problem shapes (hardcoded): votes [4, 1152, 1152, 17] f32, beta_v [1,32,1,1],
beta_a [1,32,1], output_dim=32, num_routing=3. Output [4, 32, 6, 6, 17].

Strategy: shard the output-capsule axis C=1152 across 8 cores (144 each).
Host pre-transposes each core's vote shard to a p-major (p,c8)-on-partition
fp16 layout: 18 tiles of [128 = 16 pose x 8 caps, I=1152] per batch; the
shard stays SBUF-resident across all 3 EM iterations.

Per-partition EM params (mu, -g, lnE) make the Gaussian 1-3 ops; p-sums and
row-sums are tiny shared-selector matmuls on TensorE; the q -> (c,p) row
replication is a single stride-0-partition SBUF->SBUF DMA per tile (p-major
makes the replicated view contiguous); stats products run as 2x-mode
tensor_tensor with 4x-mode tensor_scalar accumulates (a few tiles on
GpSimd for balance). One activation-table set (exp/ln/square/identity)
serves the whole kernel: sqrt -> 0.5*ln, sigmoid -> exp + tiny reciprocal.
The only cross-core exchange is the [1, I] R-normalizer row, AllReduce'd
per (batch, iteration) and hidden under the paired batch's compute.
"""

import math
import numpy as np
from contextlib import ExitStack

# ---- problem constants (hardcoded per the task contract) ----
B = 4
I_DIM = 1152
C_DIM = 1152
P_DIM = 16
D_DIM = 17
N_CORES = 8
NUM_ROUTING = 3
O_DIM = 32
WW = 36  # w*w = 6*6 positions per output capsule

CL = C_DIM // N_CORES        # 144 local caps
NT = CL * P_DIM // 128       # 18 tiles of [128, I]
NTM = 16                     # tiles whose caps fit the main [128, I] c-pack
NOV = CL - 128               # 16 overflow caps (tiles 16, 17)
NS = 3                       # i-slices per tile for PSUM bank alignment
SW = I_DIM // NS             # 384 columns per slice

EPS = 1e-10
LAMBDA = 1e-4
LN_2PI = math.log(2.0 * math.pi)

# ---- tuning knobs ----
N_ACT = 18                   # tiles using the scalar-engine Square form of p1
GP_TILES = (2, 5, 8, 11, 14, 17)           # p2 tiles whose products run on GpSimd

_NC_CACHE = {}


def _patch_tile_drain():
    """This walrus build only accepts one sync-wait on a CTRL instruction;
    spread the Tile exit-drain waits across single-wait NOPs."""
    import concourse.tile as tile
    import concourse.mybir as mybir
    from concourse.vector_clock import ScopedClock

    if getattr(tile.TileContext, "_drain_patched", False):
        return

    def _patched(self, tick_clock, wait_clock):
        nc = self.nc
        probe = nc.sync.nop()
        wait_clock.add_sem_waits(
            probe.ins, ScopedClock({None: tick_clock.global_clock})
        )
        waits = list(probe.ins.sync_info.on_wait) if probe.ins.sync_info else []
        if probe.ins.sync_info:
            probe.ins.sync_info.on_wait = waits[:1]
        for w in waits[1:]:
            n2 = nc.sync.nop()
            if n2.ins.sync_info is None:
                n2.ins.sync_info = mybir.SyncInfo(on_wait=[w], on_update=[])
            else:
                n2.ins.sync_info.on_wait = [w]
        nc.sync.drain()
        nc.all_engine_barrier()
        assert self.sems is not None
        popped = nc._tile_sem_poison_stack.pop()
        assert popped is self._sem_poison
        nc.clear_and_free_semaphores(list(self.sems.allocated().values()))
        nc.all_engine_barrier()

    tile.TileContext._drain_and_barrier = _patched
    tile.TileContext._drain_patched = True


def _split_sync_waits(nc, max_waits=1):
    """This walrus build accepts at most one sync-wait per instruction;
    move excess waits onto preceding same-engine NOPs."""
    import concourse.mybir as mybir

    uid = [0]
    for fn in nc.m.functions:
        for bb in fn.blocks:
            insts = bb.instructions
            out = []
            for inst in insts:
                si = inst.sync_info
                if si is not None and si.on_wait and len(si.on_wait) > max_waits:
                    waits = list(si.on_wait)
                    keep = waits[-max_waits:]
                    for w in waits[:-max_waits]:
                        uid[0] += 1
                        nop = mybir.InstNoOp(
                            name=f"I-waitsplit-{uid[0]}", ins=[], outs=[])
                        nop.engine = inst.engine
                        nop.sync_info = mybir.SyncInfo(on_wait=[w], on_update=[])
                        out.append(nop)
                    si.on_wait = keep
                out.append(inst)
            bb.instructions = out
    return nc


def build_nc(num_routing=NUM_ROUTING, split_waits=True):
    """Build the per-core SPMD Bass program (identical on every core)."""
    import concourse.bass as bass
    import concourse.mybir as mybir
    import concourse.tile as tile

    _patch_tile_drain()

    f32 = mybir.dt.float32
    f16 = mybir.dt.float16
    ALU = mybir.AluOpType
    ACTF = mybir.ActivationFunctionType

    nc = bass.Bass()
    vt_in = nc.declare_dram_parameter("vt", [B, NT, 128, I_DIM], f16, isOutput=False)
    at_in = nc.declare_dram_parameter("at", [B, 128, I_DIM], f16, isOutput=False)
    av_in = nc.declare_dram_parameter("av", [B, NOV, I_DIM], f16, isOutput=False)
    bdp_in = nc.declare_dram_parameter("bdp", [128, NTM * 128], f16, isOutput=False)
    bdov_in = nc.declare_dram_parameter("bdov", [128, 2 * NOV], f16, isOutput=False)
    am_in = nc.declare_dram_parameter("am", [128, 128], f16, isOutput=False)
    mm_in = nc.declare_dram_parameter("mm", [128, NT], f16, isOutput=False)
    m2_in = nc.declare_dram_parameter("m2", [NOV, 2], f16, isOutput=False)
    bd8_in = nc.declare_dram_parameter("bd8", [128, 8], f16, isOutput=False)
    bdt8_in = nc.declare_dram_parameter("bdt8", [8, 128], f16, isOutput=False)
    bv8_in = nc.declare_dram_parameter("bv8", [8, NT], f32, isOutput=False)
    ba8_in = nc.declare_dram_parameter("ba8", [8, NT], f32, isOutput=False)
    out_mu = nc.declare_dram_parameter("out_mu", [B, NT, 128], f32, isOutput=True)
    out_a = nc.declare_dram_parameter("out_a", [B, NT, 8], f32, isOutput=True)
    rs_loc = nc.dram_tensor("rs_loc", [B, I_DIM], f32)
    rs_sh = nc.dram_tensor("rs_sh", [B, I_DIM], f32, addr_space="Shared")
    qd = nc.dram_tensor("qd", [B, CL, I_DIM], f16)
    rcpd = nc.dram_tensor("rcpd", [B, 1, I_DIM], f16)

    groups = [list(range(N_CORES))]

    with tile.TileContext(nc) as tc, ExitStack() as ctx:
        pconst = ctx.enter_context(tc.tile_pool(name="const", bufs=1))
        pv = ctx.enter_context(tc.tile_pool(name="vt", bufs=2))
        pat = ctx.enter_context(tc.tile_pool(name="at", bufs=3))
        pwk = ctx.enter_context(tc.tile_pool(name="wk", bufs=2))
        pe_ = ctx.enter_context(tc.tile_pool(name="ex", bufs=2))
        pap = ctx.enter_context(tc.tile_pool(name="apS", bufs=2))
        pqc = ctx.enter_context(tc.tile_pool(name="qc", bufs=3))
        ps1 = ctx.enter_context(tc.tile_pool(name="s1o", bufs=2))
        prs = ctx.enter_context(tc.tile_pool(name="rs", bufs=2))
        psm = ctx.enter_context(tc.tile_pool(name="sm", bufs=2))
        # PSUM budget (8 banks): ap ring 3 (shared with rp) + ov 3 + q ring 2
        pps_ap = ctx.enter_context(tc.tile_pool(name="pap", bufs=1, space="PSUM"))
        pps_ov = ctx.enter_context(tc.tile_pool(name="pov", bufs=1, space="PSUM"))
        pps_q = ctx.enter_context(tc.tile_pool(name="pq", bufs=2, space="PSUM"))

        # ---- constants ----
        bdp = pconst.tile([128, NTM, 128], f16)
        nc.sync.dma_start(bdp[:].rearrange("p a b -> p (a b)"), bdp_in[:])
        bdov = pconst.tile([128, 2, NOV], f16)
        nc.sync.dma_start(bdov[:].rearrange("p a b -> p (a b)"), bdov_in[:])
        amat = pconst.tile([128, 128], f16)
        nc.sync.dma_start(amat[:], am_in[:])
        mmask = pconst.tile([128, NT], f16)
        nc.sync.dma_start(mmask[:], mm_in[:])
        m2mask = pconst.tile([NOV, 2], f16)
        nc.sync.dma_start(m2mask[:], m2_in[:])
        bd8 = pconst.tile([128, 8], f16)
        nc.sync.dma_start(bd8[:], bd8_in[:])
        bdt8 = pconst.tile([8, 128], f16)
        nc.sync.dma_start(bdt8[:], bdt8_in[:])
        bv8 = pconst.tile([8, NT], f32)
        nc.sync.dma_start(bv8[:], bv8_in[:])
        ba8 = pconst.tile([8, NT], f32)
        nc.sync.dma_start(ba8[:], ba8_in[:])
        onesA = pconst.tile([128, 1], f16)
        nc.vector.memset(onesA[:], 1.0)
        onesV = pconst.tile([NOV, 1], f16)
        nc.vector.memset(onesV[:], 1.0)
        eps_col = pconst.tile([128, 1], f32)
        nc.vector.memset(eps_col[:], EPS)

        vts, ats, avs = {}, {}, {}
        loaded = set()

        def load(b):
            if b >= B or b in loaded:
                return
            loaded.add(b)
            vt = pv.tile([128, NT, I_DIM], f16, tag="vt", name=f"vt{b}")
            nc.sync.dma_start(
                vt[:], vt_in[b].rearrange("t p i -> p t i"))
            at = pat.tile([128, I_DIM], f16, tag="at", name=f"at{b}")
            nc.sync.dma_start(at[:], at_in[b])
            av = pat.tile([NOV, I_DIM], f16, tag="av", name=f"av{b}")
            nc.sync.dma_start(av[:], av_in[b])
            vts[b], ats[b], avs[b] = vt, at, av

        params = {}   # b -> (mu, negg, lnE, sqg, bng) [128, NT] f32 tiles

        def p1_gen(b, k):
            """Gaussian weights e -> per-cap p-sum ap (f16 SBUF), local
            row-sum, AllReduce launch. Yields once per tile."""
            vt = vts[b]
            mu_t, negg_t, lnE_t, sqg_t, bng_t = params[b]
            ap_ps = pps_ap.tile([128, NS, 512], f32, tag="ap", name=f"ap{b}_{k}")
            ov_ps = pps_ov.tile([NOV, NS, 512], f32, tag="ov", name=f"ov{b}_{k}")
            for t in range(NT):
                V = vt[:, t, :]
                e = pe_.tile([128, I_DIM], f16, tag="e", name=f"e{b}{k}{t}")
                if t < N_ACT:
                    # ACT form: u = (sqrt(g) V - sqrt(g) mu)^2 ; e = exp(-u+lnE)
                    u = pwk.tile([128, I_DIM], f16, tag="d", name=f"u{b}{k}{t}")
                    nc.scalar.activation(u[:], V, ACTF.Square,
                                         bias=bng_t[:, t : t + 1],
                                         scale=sqg_t[:, t : t + 1])
                    nc.scalar.activation(e[:], u[:], ACTF.Exp,
                                         bias=lnE_t[:, t : t + 1], scale=-1.0)
                else:
                    # DVE form: d2 = (V - mu)^2 ; e = exp(negg d2 + lnE)
                    d = pwk.tile([128, I_DIM], f16, tag="d", name=f"d{b}{k}{t}")
                    nc.vector.tensor_scalar(
                        d[:], V, mu_t[:, t : t + 1], None, op0=ALU.subtract)
                    d2 = pwk.tile([128, I_DIM], f16, tag="z", name=f"d2{b}{k}{t}")
                    nc.vector.tensor_tensor(d2[:], d[:], d[:], op=ALU.mult)
                    nc.scalar.activation(e[:], d2[:], ACTF.Exp,
                                         bias=lnE_t[:, t : t + 1],
                                         scale=negg_t[:, t : t + 1])
                # p-sum: rows (p, c8) of tile t -> cap row 8t + c8
                for s in range(NS):
                    rhs = e[:, s * SW : (s + 1) * SW]
                    if t < NTM:
                        nc.tensor.matmul(ap_ps[:, s, 0:SW], bdp[:, t, :], rhs,
                                         start=(t == 0), stop=(t == NTM - 1))
                    else:
                        nc.tensor.matmul(ov_ps[:, s, 0:SW], bdov[:, t - NTM, :],
                                         rhs, start=(t == NTM), stop=(t == NT - 1))
                yield
            # free PSUM early: f16 copies of the cap-packed ap
            apS = pap.tile([128, I_DIM], f16, tag="apS", name=f"apS{b}{k}")
            nc.scalar.activation(
                apS[:].rearrange("p (s x) -> p s x", s=NS),
                ap_ps[:, :, 0:SW], ACTF.Identity)
            ovS = pap.tile([NOV, I_DIM], f16, tag="ovS", name=f"ovS{b}{k}")
            nc.scalar.activation(
                ovS[:].rearrange("p (s x) -> p s x", s=NS),
                ov_ps[:, :, 0:SW], ACTF.Identity)
            # local row-sum over all c: ones-matmuls on the f16 ap copies
            # (reuses the ap psum banks, which are dead after the copies)
            rp = pps_ap.tile([128, NS, 512], f32, tag="ap", name=f"rp{b}{k}")
            for s in range(NS):
                nc.tensor.matmul(rp[0:1, s, 0:SW], onesA[:],
                                 apS[:, s * SW : (s + 1) * SW],
                                 start=True, stop=False)
                nc.tensor.matmul(rp[0:1, s, 0:SW], onesV[:],
                                 ovS[:, s * SW : (s + 1) * SW],
                                 start=False, stop=True)
            rs_row = prs.tile([1, I_DIM], f32, tag="rsrow", name=f"rsrow{b}{k}")
            nc.scalar.activation(rs_row[:].rearrange("p (s x) -> p s x", s=NS),
                                 rp[0:1, :, 0:SW], ACTF.Identity)
            nc.sync.dma_start(rs_loc[b], rs_row[:])
            nc.gpsimd.collective_compute(
                "AllReduce", ALU.add, replica_groups=groups,
                ins=[rs_loc[b]], outs=[rs_sh[b]])
            return apS, ovS

        def qmini(b, k, apS, ovS):
            """R-normalize + vote-activation weight -> cap-packed q in SBUF."""
            # read the AllReduce'd row as [128, 9] so the iterative
            # reciprocal runs partition-parallel (~60 cyc, not ~7.5us)
            rsg = prs.tile([128, 9], f32, tag="rsg", name=f"rsg{b}{k}")
            nc.sync.dma_start(rsg[:], rs_sh[b].rearrange("(r j) -> r j", j=9))
            nc.vector.tensor_scalar(rsg[:], rsg[:], EPS, None, op0=ALU.add)
            rcpf = prs.tile([128, 9], f32, tag="rcpf", name=f"rcpf{b}{k}")
            nc.vector.reciprocal(rcpf[:], rsg[:])
            rcp1 = prs.tile([128, 9], f16, tag="rcp1", name=f"rcp1{b}{k}")
            with nc.allow_low_precision(
                    reason="f16 R-normalizer; gate tolerance 2e-2"):
                nc.vector.tensor_scalar(rcp1[:], rcpf[:], 1.0, None,
                                        op0=ALU.mult)
            # broadcast the row to all partitions via a DRAM bounce
            nc.sync.dma_start(
                rcpd[b].rearrange("o (r j) -> (o r) j", j=9), rcp1[:])
            rcp = prs.tile([128, I_DIM], f16, tag="rcp", name=f"rcp{b}{k}")
            nc.sync.dma_start(rcp[:], rcpd[b].broadcast_to((128, I_DIM)))
            qp = ps1.tile([128, I_DIM], f16, tag="qp", name=f"qp{b}{k}")
            nc.vector.tensor_tensor(qp[:], apS[:], rcp[:], op=ALU.mult)
            nc.vector.tensor_tensor(qp[:], qp[:], ats[b][:], op=ALU.mult)
            qv = ps1.tile([NOV, I_DIM], f16, tag="qv", name=f"qv{b}{k}")
            nc.vector.tensor_tensor(qv[:], ovS[:], rcp[0:NOV, :], op=ALU.mult)
            nc.vector.tensor_tensor(qv[:], qv[:], avs[b][:], op=ALU.mult)
            nc.sync.dma_start(qd[b, 0:128], qp[:])
            nc.sync.dma_start(qd[b, 128:CL], qv[:])
            return qp, qv

        def p2_gen(b, k, qp, qv):
            """Stats via DMA-broadcast q + 2x products + 4x accumulates,
            then the small per-cap math."""
            vt = vts[b]
            S1 = psm.tile([128, NT], f32, tag="S1", name=f"S1{b}{k}")
            S2 = psm.tile([128, NT], f32, tag="S2", name=f"S2{b}{k}")
            # S0 (= sum_i q) per cap via in-place identity + accum, then
            # selector-matmuls spread it to (p,c8) rows and [8, NT] form
            S0q = psm.tile([128, 1], f32, tag="S0q", name=f"S0q{b}{k}")
            nc.vector.tensor_scalar(qp[:], qp[:], 1.0, 0.0, op0=ALU.mult,
                                    op1=ALU.add, accum_out=S0q[:])
            S0v = psm.tile([NOV, 1], f32, tag="S0v", name=f"S0v{b}{k}")
            nc.vector.tensor_scalar(qv[:], qv[:], 1.0, 0.0, op0=ALU.mult,
                                    op1=ALU.add, accum_out=S0v[:])
            Bm = psm.tile([128, NT], f16, tag="Bm", name=f"Bm{b}{k}")
            nc.vector.tensor_scalar(Bm[:], mmask[:], S0q[:], None, op0=ALU.mult)
            B2 = psm.tile([NOV, 2], f16, tag="B2", name=f"B2{b}{k}")
            nc.vector.tensor_scalar(B2[:], m2mask[:], S0v[:], None, op0=ALU.mult)
            s0w_ps = pps_q.tile([128, 512], f32, tag="q", name=f"s0w{b}{k}")
            nc.tensor.matmul(s0w_ps[:, 0:NTM], amat[:], Bm[:, 0:NTM],
                             start=True, stop=True)
            nc.tensor.matmul(s0w_ps[:, NTM:NT], amat[0:NOV, :], B2[:],
                             start=True, stop=True)
            # S0 arranged [8, NT] for the per-cap cost math
            s08_ps = pps_q.tile([128, 512], f32, tag="q", name=f"s08{b}{k}")
            nc.tensor.matmul(s08_ps[0:8, 0:NTM], amat[:, 0:8], Bm[:, 0:NTM],
                             start=True, stop=True)
            nc.tensor.matmul(s08_ps[0:8, NTM:NT], amat[0:NOV, 0:8], B2[:],
                             start=True, stop=True)

            for t in range(NT):
                V = vt[:, t, :]
                qcp = pqc.tile([128, I_DIM], f16, tag="qc", name=f"qc{b}{k}{t}")
                if k == 0:
                    if t < NTM:
                        src = at_in[b, 8 * t : 8 * t + 8, :]
                    else:
                        src = av_in[b, 8 * (t - NTM) : 8 * (t - NTM) + 8, :]
                else:
                    if t < NTM:
                        src = qd[b, 8 * t : 8 * t + 8, :]
                    else:
                        src = qd[b, 128 + 8 * (t - NTM) :
                                 128 + 8 * (t - NTM) + 8, :]
                nc.sync.dma_start(qcp[:], src.partition_broadcast(16))
                s1o = ps1.tile([128, I_DIM], f16, tag="s1o", name=f"s1o{b}{k}{t}")
                s2o = ps1.tile([128, I_DIM], f16, tag="s2o", name=f"s2o{b}{k}{t}")
                # DVE accumulation is 1x-only, so the fused STT (product +
                # accum in one pass) is optimal; spill some tiles' s1o to
                # GpSimd product + scalar-engine accumulate for balance.
                if t in GP_TILES:
                    nc.gpsimd.tensor_tensor(s1o[:], qcp[:], V, op=ALU.mult)
                    nc.scalar.activation(s1o[:], s1o[:], ACTF.Identity,
                                         accum_out=S1[:, t : t + 1])
                else:
                    nc.vector.scalar_tensor_tensor(
                        s1o[:], qcp[:], 1.0, V, op0=ALU.mult, op1=ALU.mult,
                        accum_out=S1[:, t : t + 1])
                nc.vector.scalar_tensor_tensor(
                    s2o[:], s1o[:], 1.0, V, op0=ALU.mult, op1=ALU.mult,
                    accum_out=S2[:, t : t + 1])
                yield

            # ---- small math on [128, NT] f32 (p-major rows) ----
            rS = psm.tile([128, NT], f32, tag="rS", name=f"rS{b}{k}")
            nc.vector.reciprocal(rS[:], s0w_ps[:, 0:NT])
            mu = psm.tile([128, NT], f32, tag="mu", name=f"mu{b}{k}")
            nc.vector.tensor_tensor(mu[:], S1[:], rS[:], op=ALU.mult)
            ex2 = psm.tile([128, NT], f32, tag="ex2", name=f"ex2{b}{k}")
            nc.vector.tensor_tensor(ex2[:], S2[:], rS[:], op=ALU.mult)
            mu2 = psm.tile([128, NT], f32, tag="mu2", name=f"mu2{b}{k}")
            nc.vector.tensor_tensor(mu2[:], mu[:], mu[:], op=ALU.mult)
            sig2 = psm.tile([128, NT], f32, tag="sig2", name=f"sig2{b}{k}")
            nc.vector.tensor_tensor(sig2[:], ex2[:], mu2[:], op=ALU.subtract)
            nc.vector.tensor_scalar_max(sig2[:], sig2[:], 1e-12)
            # log sigma = 0.5 ln(sig2); the 0.5 is folded into bd8/lnE uses
            L = psm.tile([128, NT], f16, tag="L", name=f"L{b}{k}")
            nc.scalar.activation(L[:], sig2[:], ACTF.Ln)
            # per-cap cost: smp[c8, t] = sum_p L ; bd8 entries are 0.5
            smp = pps_q.tile([128, 512], f32, tag="q", name=f"smp{b}{k}")
            nc.tensor.matmul(smp[0:8, 0:NT], bd8[:], L[:], start=True, stop=True)
            c1 = psm.tile([8, NT], f32, tag="c1", name=f"c1{b}{k}")
            nc.vector.tensor_tensor(c1[:], smp[0:8, 0:NT], bv8[:], op=ALU.add)
            c2 = psm.tile([8, NT], f32, tag="c2", name=f"c2{b}{k}")
            nc.vector.tensor_tensor(c2[:], c1[:], s08_ps[0:8, 0:NT], op=ALU.mult)
            wk = (1.0 / O_DIM) if k == 0 else 1.0
            ain = psm.tile([8, NT], f32, tag="ain", name=f"ain{b}{k}")
            nc.vector.scalar_tensor_tensor(
                ain[:], c2[:], -wk, ba8[:], op0=ALU.mult, op1=ALU.add)
            # a = sigmoid(LAMBDA * ain) = 1 / (1 + exp(-LAMBDA * ain))
            ea = psm.tile([8, NT], f32, tag="ea", name=f"ea{b}{k}")
            nc.scalar.activation(ea[:], ain[:], ACTF.Exp, scale=-LAMBDA)
            ua = psm.tile([8, NT], f32, tag="ua", name=f"ua{b}{k}")
            nc.vector.tensor_scalar(ua[:], ea[:], 1.0, None, op0=ALU.add)
            a8 = psm.tile([8, NT], f32, tag="a8", name=f"a8{b}{k}")
            nc.vector.reciprocal(a8[:], ua[:])

            if k == num_routing - 1:
                nc.sync.dma_start(out_mu[b].rearrange("t r -> r t"), mu[:])
                nc.sync.dma_start(out_a[b].rearrange("t c -> c t"), a8[:])
            else:
                a816 = psm.tile([8, NT], f16, tag="a816", name=f"a816{b}{k}")
                with nc.allow_low_precision(reason="a broadcast; tol 2e-2"):
                    nc.vector.tensor_scalar(a816[:], a8[:], 1.0, None,
                                            op0=ALU.mult)
                arep = pps_q.tile([128, 512], f32, tag="q", name=f"ar{b}{k}")
                nc.tensor.matmul(arep[:, 0:NT], bdt8[:], a816[:],
                                 start=True, stop=True)
                lnA = psm.tile([128, NT], f32, tag="lnA", name=f"lnA{b}{k}")
                nc.scalar.activation(lnA[:], arep[:, 0:NT], ACTF.Ln, bias=eps_col[:])
                # lnE = lnA - 0.5 ln(sig2) - 0.5 ln(2pi)
                lnE = psm.tile([128, NT], f32, tag="lnE", name=f"lnE{b}{k}")
                nc.vector.scalar_tensor_tensor(
                    lnE[:], L[:], -0.5, lnA[:], op0=ALU.mult, op1=ALU.add)
                nc.vector.tensor_scalar(lnE[:], lnE[:], -0.5 * LN_2PI, None,
                                        op0=ALU.add)
                rsig = psm.tile([128, NT], f32, tag="rsig", name=f"rv{b}{k}")
                nc.vector.reciprocal(rsig[:], sig2[:])
                negg = psm.tile([128, NT], f32, tag="negg", name=f"ng{b}{k}")
                nc.vector.tensor_scalar_mul(negg[:], rsig[:], -0.5)
                if N_ACT > 0:
                    # sqrt(g) = exp(0.5 ln(0.5 rsig)); bng = -sqrt(g) mu
                    lng = psm.tile([128, NT], f32, tag="lng", name=f"lg{b}{k}")
                    nc.scalar.activation(lng[:], rsig[:], ACTF.Ln, scale=0.5)
                    sqg = psm.tile([128, NT], f32, tag="sqg", name=f"sq{b}{k}")
                    nc.scalar.activation(sqg[:], lng[:], ACTF.Exp, scale=0.5)
                    bng = psm.tile([128, NT], f32, tag="bng", name=f"bg{b}{k}")
                    nc.vector.scalar_tensor_tensor(
                        bng[:], sqg[:], -1.0, mu[:], op0=ALU.mult, op1=ALU.mult)
                else:
                    sqg = bng = negg
                params[b] = (mu, negg, lnE, sqg, bng)

        # ---------------- schedule: batch pairs, AR hidden ----------------
        def drain(g):
            r = None
            while True:
                try:
                    r = next(g)
                except StopIteration as e:
                    return e.value

        for b0 in range(0, B, 2):
            b1 = b0 + 1
            load(b0)
            load(b1)
            load(b0 + 2)
            drain(p2_gen(b0, 0, ats[b0], avs[b0]))
            drain(p2_gen(b1, 0, ats[b1], avs[b1]))
            for k in range(1, num_routing):
                h0 = drain(p1_gen(b0, k))
                h1 = drain(p1_gen(b1, k))
                q0 = qmini(b0, k, *h0)
                drain(p2_gen(b0, k, *q0))
                q1 = qmini(b1, k, *h1)
                drain(p2_gen(b1, k, *q1))

    if split_waits:
        _split_sync_waits(nc)
    return nc


# ------------------------- host-side wrapper ----------------------------

def make_consts():
    """Selector/mask constants for the p-major (p, c8) packing."""
    # p-sum selectors: rows (8p + c8) of tile t -> cap col 8t + c8
    bdp = np.zeros((128, NTM, 128), np.float16)
    for t in range(NTM):
        for p in range(16):
            for c8 in range(8):
                bdp[8 * p + c8, t, 8 * t + c8] = 1.0
    bdov = np.zeros((128, 2, NOV), np.float16)
    for tv in range(2):
        for p in range(16):
            for c8 in range(8):
                bdov[8 * p + c8, tv, 8 * tv + c8] = 1.0
    # S0 spread: A[k, r] = 1 iff k % 8 == r % 8
    amat = np.zeros((128, 128), np.float16)
    for kk in range(128):
        for r in range(kk % 8, 128, 8):
            amat[kk, r] = 1.0
    # tile masks: M[k, t] = 1 iff k // 8 == t
    mmask = np.zeros((128, NT), np.float16)
    for kk in range(128):
        mmask[kk, kk // 8] = 1.0
    m2mask = np.zeros((NOV, 2), np.float16)
    for kk in range(NOV):
        m2mask[kk, kk // 8] = 1.0
    # p-reduce within cap, folded 0.5 for log sigma = 0.5 ln sig2
    bd8 = np.zeros((128, 8), np.float16)
    for p in range(16):
        for c8 in range(8):
            bd8[8 * p + c8, c8] = 0.5
    # a broadcast: [8, NT] -> (p, c8) rows
    bdt8 = np.zeros((8, 128), np.float16)
    for c8 in range(8):
        for p in range(16):
            bdt8[c8, 8 * p + c8] = 1.0
    return bdp, bdov, amat, mmask, m2mask, bd8, bdt8


def _get_nc():
    key = "full"
    if key not in _NC_CACHE:
        _NC_CACHE[key] = build_nc()
    return _NC_CACHE[key]


def make_in_maps(votes, beta_v, beta_a):
    """votes [B, I, C, D] f32 -> per-core input dicts (p-major packing)."""
    bvc = 16.0 * np.repeat(beta_v.reshape(-1), WW)   # [C], pre-scaled by P
    bac = np.repeat(beta_a.reshape(-1), WW)
    bdp_np, bdov_np, am_np, mm_np, m2_np, bd8_np, bdt8_np = make_consts()
    vt_all = np.ascontiguousarray(votes.transpose(0, 2, 3, 1))  # [B, C, D, I]
    in_maps = []
    for c in range(N_CORES):
        sl = slice(c * CL, (c + 1) * CL)
        blk = vt_all[:, sl]                               # [B, CL, D, I]
        pose = blk[:, :, :P_DIM, :].astype(np.float16)    # [B, CL, 16, I]
        # [B, CL=18*8 caps, 16 pose, I] -> [B, t, p, c8, i] -> [B,t,8p+c8,i]
        vt = np.ascontiguousarray(
            pose.reshape(B, NT, 8, P_DIM, I_DIM).transpose(0, 1, 3, 2, 4)
            .reshape(B, NT, 128, I_DIM))
        acts = blk[:, :, P_DIM, :].astype(np.float16)     # [B, CL, I]
        at = np.ascontiguousarray(acts[:, :128, :])
        av = np.ascontiguousarray(acts[:, 128:, :])
        # per-cap consts in [c8, t] layout: col t, row c8 -> cap 8t + c8
        cl_idx = np.arange(CL)
        bv8 = np.ascontiguousarray(
            bvc[c * CL + cl_idx].reshape(NT, 8).T.astype(np.float32))
        ba8 = np.ascontiguousarray(
            bac[c * CL + cl_idx].reshape(NT, 8).T.astype(np.float32))
        in_maps.append({
            "vt": vt, "at": at, "av": av,
            "bdp": np.ascontiguousarray(bdp_np.reshape(128, NTM * 128)),
            "bdov": np.ascontiguousarray(bdov_np.reshape(128, 2 * NOV)),
            "am": am_np, "mm": mm_np, "m2": m2_np,
            "bd8": bd8_np, "bdt8": bdt8_np,
            "bv8": bv8, "ba8": ba8,
        })
    return in_maps


def assemble_output(results):
    """Per-core out_mu [B, NT, 128] + out_a [B, NT, 8] -> [B, O, w, w, D].

    out_mu rows are p-major: row (8p + c8) of tile t = cap 8t+c8, pose p.
    """
    full = np.zeros((B, C_DIM, D_DIM), np.float32)
    for c in range(N_CORES):
        om = np.asarray(results[c]["out_mu"])             # [B, NT, 128]
        oa = np.asarray(results[c]["out_a"])              # [B, NT, 8]
        sl = slice(c * CL, (c + 1) * CL)
        # [B, t, (p, c8)] -> [B, t, c8, p] -> [B, CL, P]
        mu = om.reshape(B, NT, P_DIM, 8).transpose(0, 1, 3, 2)
        full[:, sl, :P_DIM] = mu.reshape(B, CL, P_DIM)
        full[:, sl, P_DIM] = oa.reshape(B, CL)
    w = int(math.sqrt(C_DIM // O_DIM))
    return full.reshape(B, O_DIM, w, w, D_DIM).astype(np.float32)


def kernel(**inputs) -> np.ndarray:
    from concourse.bass_utils import run_bass_kernel_spmd

    votes = np.ascontiguousarray(np.asarray(inputs["votes"], dtype=np.float32))
    beta_v = np.asarray(inputs["beta_v"], dtype=np.float32)
    beta_a = np.asarray(inputs["beta_a"], dtype=np.float32)
    output_dim = int(np.asarray(inputs["output_dim"]))
    num_routing = int(np.asarray(inputs["num_routing"]))
    assert votes.shape == (B, I_DIM, C_DIM, D_DIM), votes.shape
    assert output_dim == O_DIM and num_routing == NUM_ROUTING

    nc = _get_nc()
    in_maps = make_in_maps(votes, beta_v, beta_a)
    res = run_bass_kernel_spmd(nc, in_maps, list(range(N_CORES)))
    return assemble_output([res.results[i] for i in range(N_CORES)])


# revision 25
# speedup vs baseline: 1.2010x; 1.0960x over previous
"""Trainium2 Bass kernel for EM matrix-capsule routing (nn_MatrixRouting).

Problem shapes (hardcoded): votes [4, 1152, 1152, 17] f32, beta_v [1,32,1,1],
beta_a [1,32,1], output_dim=32, num_routing=3. Output [4, 32, 6, 6, 17].

Strategy: shard the output-capsule axis C=1152 across 8 cores (144 each).
Host pre-transposes each core's vote shard to a p-major (p,c8)-on-partition
fp16 layout: 18 tiles of [128 = 16 pose x 8 caps, I=1152] per batch; the
shard stays SBUF-resident across all 3 EM iterations.

Per-partition EM params (mu, -g, lnE) make the Gaussian 1-3 ops; p-sums and
row-sums are tiny shared-selector matmuls on TensorE; the q -> (c,p) row
replication is a single stride-0-partition SBUF->SBUF DMA per tile (p-major
makes the replicated view contiguous); stats products run as 2x-mode
tensor_tensor with 4x-mode tensor_scalar accumulates (a few tiles on
GpSimd for balance). One activation-table set (exp/ln/square/identity)
serves the whole kernel: sqrt -> 0.5*ln, sigmoid -> exp + tiny reciprocal.
The only cross-core exchange is the [1, I] R-normalizer row, AllReduce'd
per (batch, iteration) and hidden under the paired batch's compute.
"""

import math
import numpy as np
from contextlib import ExitStack

# ---- problem constants (hardcoded per the task contract) ----
B = 4
I_DIM = 1152
C_DIM = 1152
P_DIM = 16
D_DIM = 17
N_CORES = 8
NUM_ROUTING = 3
O_DIM = 32
WW = 36  # w*w = 6*6 positions per output capsule

CL = C_DIM // N_CORES        # 144 local caps
NT = CL * P_DIM // 128       # 18 tiles of [128, I]
NTM = 16                     # tiles whose caps fit the main [128, I] c-pack
NOV = CL - 128               # 16 overflow caps (tiles 16, 17)
NS = 3                       # i-slices per tile for PSUM bank alignment
SW = I_DIM // NS             # 384 columns per slice

EPS = 1e-10
LAMBDA = 1e-4
LN_2PI = math.log(2.0 * math.pi)

# ---- tuning knobs ----
N_ACT = 14                   # tiles using the scalar-engine Square form of p1
GP_TILES = ()           # p2 tiles whose products run on GpSimd

_NC_CACHE = {}


def _patch_tile_drain():
    """This walrus build only accepts one sync-wait on a CTRL instruction;
    spread the Tile exit-drain waits across single-wait NOPs."""
    import concourse.tile as tile
    import concourse.mybir as mybir
    from concourse.vector_clock import ScopedClock

    if getattr(tile.TileContext, "_drain_patched", False):
        return

    def _patched(self, tick_clock, wait_clock):
        nc = self.nc
        probe = nc.sync.nop()
        wait_clock.add_sem_waits(
            probe.ins, ScopedClock({None: tick_clock.global_clock})
        )
        waits = list(probe.ins.sync_info.on_wait) if probe.ins.sync_info else []
        if probe.ins.sync_info:
            probe.ins.sync_info.on_wait = waits[:1]
        for w in waits[1:]:
            n2 = nc.sync.nop()
            if n2.ins.sync_info is None:
                n2.ins.sync_info = mybir.SyncInfo(on_wait=[w], on_update=[])
            else:
                n2.ins.sync_info.on_wait = [w]
        nc.sync.drain()
        nc.all_engine_barrier()
        assert self.sems is not None
        popped = nc._tile_sem_poison_stack.pop()
        assert popped is self._sem_poison
        nc.clear_and_free_semaphores(list(self.sems.allocated().values()))
        nc.all_engine_barrier()

    tile.TileContext._drain_and_barrier = _patched
    tile.TileContext._drain_patched = True


def _split_sync_waits(nc, max_waits=1):
    """This walrus build accepts at most one sync-wait per instruction;
    move excess waits onto preceding same-engine NOPs."""
    import concourse.mybir as mybir

    uid = [0]
    for fn in nc.m.functions:
        for bb in fn.blocks:
            insts = bb.instructions
            out = []
            for inst in insts:
                si = inst.sync_info
                if si is not None and si.on_wait and len(si.on_wait) > max_waits:
                    waits = list(si.on_wait)
                    keep = waits[-max_waits:]
                    for w in waits[:-max_waits]:
                        uid[0] += 1
                        nop = mybir.InstNoOp(
                            name=f"I-waitsplit-{uid[0]}", ins=[], outs=[])
                        nop.engine = inst.engine
                        nop.sync_info = mybir.SyncInfo(on_wait=[w], on_update=[])
                        out.append(nop)
                    si.on_wait = keep
                out.append(inst)
            bb.instructions = out
    return nc


def build_nc(num_routing=NUM_ROUTING, split_waits=True):
    """Build the per-core SPMD Bass program (identical on every core)."""
    import concourse.bass as bass
    import concourse.mybir as mybir
    import concourse.tile as tile

    _patch_tile_drain()

    f32 = mybir.dt.float32
    f16 = mybir.dt.float16
    ALU = mybir.AluOpType
    ACTF = mybir.ActivationFunctionType

    nc = bass.Bass()
    vt_in = nc.declare_dram_parameter("vt", [B, NT, 128, I_DIM], f16, isOutput=False)
    at_in = nc.declare_dram_parameter("at", [B, 128, I_DIM], f16, isOutput=False)
    av_in = nc.declare_dram_parameter("av", [B, NOV, I_DIM], f16, isOutput=False)
    bdp_in = nc.declare_dram_parameter("bdp", [128, NTM * 128], f16, isOutput=False)
    bdov_in = nc.declare_dram_parameter("bdov", [128, 2 * NOV], f16, isOutput=False)
    am_in = nc.declare_dram_parameter("am", [128, 128], f16, isOutput=False)
    mm_in = nc.declare_dram_parameter("mm", [128, NT], f16, isOutput=False)
    m2_in = nc.declare_dram_parameter("m2", [NOV, 2], f16, isOutput=False)
    bd8_in = nc.declare_dram_parameter("bd8", [128, 8], f16, isOutput=False)
    bdt8_in = nc.declare_dram_parameter("bdt8", [8, 128], f16, isOutput=False)
    bv8_in = nc.declare_dram_parameter("bv8", [8, NT], f32, isOutput=False)
    ba8_in = nc.declare_dram_parameter("ba8", [8, NT], f32, isOutput=False)
    out_mu = nc.declare_dram_parameter("out_mu", [B, NT, 128], f32, isOutput=True)
    out_a = nc.declare_dram_parameter("out_a", [B, NT, 8], f32, isOutput=True)
    rs_loc = nc.dram_tensor("rs_loc", [B, I_DIM], f32)
    rs_sh = nc.dram_tensor("rs_sh", [B, I_DIM], f32, addr_space="Shared")
    qd = nc.dram_tensor("qd", [B, CL, I_DIM], f16)
    rcpd = nc.dram_tensor("rcpd", [B, 1, I_DIM], f16)

    groups = [list(range(N_CORES))]

    with tile.TileContext(nc) as tc, ExitStack() as ctx:
        pconst = ctx.enter_context(tc.tile_pool(name="const", bufs=1))
        pv = ctx.enter_context(tc.tile_pool(name="vt", bufs=2))
        pat = ctx.enter_context(tc.tile_pool(name="at", bufs=3))
        pwk = ctx.enter_context(tc.tile_pool(name="wk", bufs=2))
        pe_ = ctx.enter_context(tc.tile_pool(name="ex", bufs=2))
        pap = ctx.enter_context(tc.tile_pool(name="apS", bufs=2))
        pqc = ctx.enter_context(tc.tile_pool(name="qc", bufs=3))
        ps1 = ctx.enter_context(tc.tile_pool(name="s1o", bufs=2))
        prs = ctx.enter_context(tc.tile_pool(name="rs", bufs=2))
        psm = ctx.enter_context(tc.tile_pool(name="sm", bufs=2))
        # PSUM budget (8 banks): ap ring 3 (shared with rp) + ov 3 + q ring 2
        pps_ap = ctx.enter_context(tc.tile_pool(name="pap", bufs=1, space="PSUM"))
        pps_ov = ctx.enter_context(tc.tile_pool(name="pov", bufs=1, space="PSUM"))
        pps_q = ctx.enter_context(tc.tile_pool(name="pq", bufs=2, space="PSUM"))

        # ---- constants ----
        bdp = pconst.tile([128, NTM, 128], f16)
        nc.sync.dma_start(bdp[:].rearrange("p a b -> p (a b)"), bdp_in[:])
        bdov = pconst.tile([128, 2, NOV], f16)
        nc.sync.dma_start(bdov[:].rearrange("p a b -> p (a b)"), bdov_in[:])
        amat = pconst.tile([128, 128], f16)
        nc.sync.dma_start(amat[:], am_in[:])
        mmask = pconst.tile([128, NT], f16)
        nc.sync.dma_start(mmask[:], mm_in[:])
        m2mask = pconst.tile([NOV, 2], f16)
        nc.sync.dma_start(m2mask[:], m2_in[:])
        bd8 = pconst.tile([128, 8], f16)
        nc.sync.dma_start(bd8[:], bd8_in[:])
        bdt8 = pconst.tile([8, 128], f16)
        nc.sync.dma_start(bdt8[:], bdt8_in[:])
        bv8 = pconst.tile([8, NT], f32)
        nc.sync.dma_start(bv8[:], bv8_in[:])
        ba8 = pconst.tile([8, NT], f32)
        nc.sync.dma_start(ba8[:], ba8_in[:])
        onesA = pconst.tile([128, 1], f16)
        nc.vector.memset(onesA[:], 1.0)
        onesV = pconst.tile([NOV, 1], f16)
        nc.vector.memset(onesV[:], 1.0)
        eps_col = pconst.tile([128, 1], f32)
        nc.vector.memset(eps_col[:], EPS)

        vts, ats, avs = {}, {}, {}
        loaded = set()

        def load(b):
            if b >= B or b in loaded:
                return
            loaded.add(b)
            vt = pv.tile([128, NT, I_DIM], f16, tag="vt", name=f"vt{b}")
            nc.sync.dma_start(
                vt[:], vt_in[b].rearrange("t p i -> p t i"))
            at = pat.tile([128, I_DIM], f16, tag="at", name=f"at{b}")
            nc.sync.dma_start(at[:], at_in[b])
            av = pat.tile([NOV, I_DIM], f16, tag="av", name=f"av{b}")
            nc.sync.dma_start(av[:], av_in[b])
            vts[b], ats[b], avs[b] = vt, at, av

        params = {}   # b -> (mu, negg, lnE, sqg, bng) [128, NT] f32 tiles

        def p1_gen(b, k):
            """Gaussian weights e -> per-cap p-sum ap (f16 SBUF), local
            row-sum, AllReduce launch. Yields once per tile."""
            vt = vts[b]
            mu_t, negg_t, lnE_t, sqg_t, bng_t = params[b]
            ap_ps = pps_ap.tile([128, NS, 512], f32, tag="ap", name=f"ap{b}_{k}")
            ov_ps = pps_ov.tile([NOV, NS, 512], f32, tag="ov", name=f"ov{b}_{k}")
            for t in range(NT):
                V = vt[:, t, :]
                e = pe_.tile([128, I_DIM], f16, tag="e", name=f"e{b}{k}{t}")
                if t < N_ACT:
                    # ACT form: u = (sqrt(g) V - sqrt(g) mu)^2 ; e = exp(-u+lnE)
                    u = pwk.tile([128, I_DIM], f16, tag="d", name=f"u{b}{k}{t}")
                    nc.scalar.activation(u[:], V, ACTF.Square,
                                         bias=bng_t[:, t : t + 1],
                                         scale=sqg_t[:, t : t + 1])
                    nc.scalar.activation(e[:], u[:], ACTF.Exp,
                                         bias=lnE_t[:, t : t + 1], scale=-1.0)
                else:
                    # DVE form: d2 = (V - mu)^2 ; e = exp(negg d2 + lnE)
                    d = pwk.tile([128, I_DIM], f16, tag="d", name=f"d{b}{k}{t}")
                    nc.vector.tensor_scalar(
                        d[:], V, mu_t[:, t : t + 1], None, op0=ALU.subtract)
                    d2 = pwk.tile([128, I_DIM], f16, tag="z", name=f"d2{b}{k}{t}")
                    nc.vector.tensor_tensor(d2[:], d[:], d[:], op=ALU.mult)
                    nc.scalar.activation(e[:], d2[:], ACTF.Exp,
                                         bias=lnE_t[:, t : t + 1],
                                         scale=negg_t[:, t : t + 1])
                # p-sum: rows (p, c8) of tile t -> cap row 8t + c8
                for s in range(NS):
                    rhs = e[:, s * SW : (s + 1) * SW]
                    if t < NTM:
                        nc.tensor.matmul(ap_ps[:, s, 0:SW], bdp[:, t, :], rhs,
                                         start=(t == 0), stop=(t == NTM - 1))
                    else:
                        nc.tensor.matmul(ov_ps[:, s, 0:SW], bdov[:, t - NTM, :],
                                         rhs, start=(t == NTM), stop=(t == NT - 1))
                yield
            # free PSUM early: f16 copies of the cap-packed ap
            apS = pap.tile([128, I_DIM], f16, tag="apS", name=f"apS{b}{k}")
            nc.scalar.activation(
                apS[:].rearrange("p (s x) -> p s x", s=NS),
                ap_ps[:, :, 0:SW], ACTF.Identity)
            ovS = pap.tile([NOV, I_DIM], f16, tag="ovS", name=f"ovS{b}{k}")
            nc.scalar.activation(
                ovS[:].rearrange("p (s x) -> p s x", s=NS),
                ov_ps[:, :, 0:SW], ACTF.Identity)
            # local row-sum over all c: ones-matmuls on the f16 ap copies
            # (reuses the ap psum banks, which are dead after the copies)
            rp = pps_ap.tile([128, NS, 512], f32, tag="ap", name=f"rp{b}{k}")
            for s in range(NS):
                nc.tensor.matmul(rp[0:1, s, 0:SW], onesA[:],
                                 apS[:, s * SW : (s + 1) * SW],
                                 start=True, stop=False)
                nc.tensor.matmul(rp[0:1, s, 0:SW], onesV[:],
                                 ovS[:, s * SW : (s + 1) * SW],
                                 start=False, stop=True)
            rs_row = prs.tile([1, I_DIM], f32, tag="rsrow", name=f"rsrow{b}{k}")
            nc.scalar.activation(rs_row[:].rearrange("p (s x) -> p s x", s=NS),
                                 rp[0:1, :, 0:SW], ACTF.Identity)
            nc.sync.dma_start(rs_loc[b], rs_row[:])
            nc.gpsimd.collective_compute(
                "AllReduce", ALU.add, replica_groups=groups,
                ins=[rs_loc[b]], outs=[rs_sh[b]])
            return apS, ovS

        def qmini(b, k, apS, ovS):
            """R-normalize + vote-activation weight -> cap-packed q in SBUF."""
            # read the AllReduce'd row as [128, 9] so the iterative
            # reciprocal runs partition-parallel (~60 cyc, not ~7.5us)
            rsg = prs.tile([128, 9], f32, tag="rsg", name=f"rsg{b}{k}")
            nc.sync.dma_start(rsg[:], rs_sh[b].rearrange("(r j) -> r j", j=9))
            nc.vector.tensor_scalar(rsg[:], rsg[:], EPS, None, op0=ALU.add)
            rcpf = prs.tile([128, 9], f32, tag="rcpf", name=f"rcpf{b}{k}")
            nc.vector.reciprocal(rcpf[:], rsg[:])
            rcp1 = prs.tile([128, 9], f16, tag="rcp1", name=f"rcp1{b}{k}")
            with nc.allow_low_precision(
                    reason="f16 R-normalizer; gate tolerance 2e-2"):
                nc.vector.tensor_scalar(rcp1[:], rcpf[:], 1.0, None,
                                        op0=ALU.mult)
            # broadcast the row to all partitions via a DRAM bounce
            nc.sync.dma_start(
                rcpd[b].rearrange("o (r j) -> (o r) j", j=9), rcp1[:])
            rcp = prs.tile([128, I_DIM], f16, tag="rcp", name=f"rcp{b}{k}")
            nc.sync.dma_start(rcp[:], rcpd[b].broadcast_to((128, I_DIM)))
            qp = ps1.tile([128, I_DIM], f16, tag="qp", name=f"qp{b}{k}")
            nc.vector.tensor_tensor(qp[:], apS[:], rcp[:], op=ALU.mult)
            nc.vector.tensor_tensor(qp[:], qp[:], ats[b][:], op=ALU.mult)
            qv = ps1.tile([NOV, I_DIM], f16, tag="qv", name=f"qv{b}{k}")
            nc.vector.tensor_tensor(qv[:], ovS[:], rcp[0:NOV, :], op=ALU.mult)
            nc.vector.tensor_tensor(qv[:], qv[:], avs[b][:], op=ALU.mult)
            nc.sync.dma_start(qd[b, 0:128], qp[:])
            nc.sync.dma_start(qd[b, 128:CL], qv[:])
            return qp, qv

        def p2_gen(b, k, qp, qv):
            """Stats via DMA-broadcast q + 2x products + 4x accumulates,
            then the small per-cap math."""
            vt = vts[b]
            S1 = psm.tile([128, NT], f32, tag="S1", name=f"S1{b}{k}")
            S2 = psm.tile([128, NT], f32, tag="S2", name=f"S2{b}{k}")
            # S0 (= sum_i q) per cap via in-place identity + accum, then
            # selector-matmuls spread it to (p,c8) rows and [8, NT] form
            S0q = psm.tile([128, 1], f32, tag="S0q", name=f"S0q{b}{k}")
            nc.vector.tensor_scalar(qp[:], qp[:], 1.0, 0.0, op0=ALU.mult,
                                    op1=ALU.add, accum_out=S0q[:])
            S0v = psm.tile([NOV, 1], f32, tag="S0v", name=f"S0v{b}{k}")
            nc.vector.tensor_scalar(qv[:], qv[:], 1.0, 0.0, op0=ALU.mult,
                                    op1=ALU.add, accum_out=S0v[:])
            Bm = psm.tile([128, NT], f16, tag="Bm", name=f"Bm{b}{k}")
            nc.vector.tensor_scalar(Bm[:], mmask[:], S0q[:], None, op0=ALU.mult)
            B2 = psm.tile([NOV, 2], f16, tag="B2", name=f"B2{b}{k}")
            nc.vector.tensor_scalar(B2[:], m2mask[:], S0v[:], None, op0=ALU.mult)
            s0w_ps = pps_q.tile([128, 512], f32, tag="q", name=f"s0w{b}{k}")
            nc.tensor.matmul(s0w_ps[:, 0:NTM], amat[:], Bm[:, 0:NTM],
                             start=True, stop=True)
            nc.tensor.matmul(s0w_ps[:, NTM:NT], amat[0:NOV, :], B2[:],
                             start=True, stop=True)
            # S0 arranged [8, NT] for the per-cap cost math
            s08_ps = pps_q.tile([128, 512], f32, tag="q", name=f"s08{b}{k}")
            nc.tensor.matmul(s08_ps[0:8, 0:NTM], amat[:, 0:8], Bm[:, 0:NTM],
                             start=True, stop=True)
            nc.tensor.matmul(s08_ps[0:8, NTM:NT], amat[0:NOV, 0:8], B2[:],
                             start=True, stop=True)

            for t in range(NT):
                V = vt[:, t, :]
                qcp = pqc.tile([128, I_DIM], f16, tag="qc", name=f"qc{b}{k}{t}")
                if k == 0:
                    if t < NTM:
                        src = at_in[b, 8 * t : 8 * t + 8, :]
                    else:
                        src = av_in[b, 8 * (t - NTM) : 8 * (t - NTM) + 8, :]
                else:
                    if t < NTM:
                        src = qd[b, 8 * t : 8 * t + 8, :]
                    else:
                        src = qd[b, 128 + 8 * (t - NTM) :
                                 128 + 8 * (t - NTM) + 8, :]
                nc.sync.dma_start(qcp[:], src.partition_broadcast(16))
                s1o = ps1.tile([128, I_DIM], f16, tag="s1o", name=f"s1o{b}{k}{t}")
                s2o = ps1.tile([128, I_DIM], f16, tag="s2o", name=f"s2o{b}{k}{t}")
                # DVE accumulation is 1x-only, so the fused STT (product +
                # accum in one pass) is optimal; spill some tiles' s1o to
                # GpSimd product + scalar-engine accumulate for balance.
                if t in GP_TILES:
                    nc.gpsimd.tensor_tensor(s1o[:], qcp[:], V, op=ALU.mult)
                    nc.scalar.activation(s1o[:], s1o[:], ACTF.Identity,
                                         accum_out=S1[:, t : t + 1])
                else:
                    nc.vector.scalar_tensor_tensor(
                        s1o[:], qcp[:], 1.0, V, op0=ALU.mult, op1=ALU.mult,
                        accum_out=S1[:, t : t + 1])
                nc.vector.scalar_tensor_tensor(
                    s2o[:], s1o[:], 1.0, V, op0=ALU.mult, op1=ALU.mult,
                    accum_out=S2[:, t : t + 1])
                yield

            # ---- small math on [128, NT] f32 (p-major rows) ----
            rS = psm.tile([128, NT], f32, tag="rS", name=f"rS{b}{k}")
            nc.vector.reciprocal(rS[:], s0w_ps[:, 0:NT])
            mu = psm.tile([128, NT], f32, tag="mu", name=f"mu{b}{k}")
            nc.vector.tensor_tensor(mu[:], S1[:], rS[:], op=ALU.mult)
            ex2 = psm.tile([128, NT], f32, tag="ex2", name=f"ex2{b}{k}")
            nc.vector.tensor_tensor(ex2[:], S2[:], rS[:], op=ALU.mult)
            mu2 = psm.tile([128, NT], f32, tag="mu2", name=f"mu2{b}{k}")
            nc.vector.tensor_tensor(mu2[:], mu[:], mu[:], op=ALU.mult)
            sig2 = psm.tile([128, NT], f32, tag="sig2", name=f"sig2{b}{k}")
            nc.vector.tensor_tensor(sig2[:], ex2[:], mu2[:], op=ALU.subtract)
            nc.vector.tensor_scalar_max(sig2[:], sig2[:], 1e-12)
            # log sigma = 0.5 ln(sig2); the 0.5 is folded into bd8/lnE uses
            L = psm.tile([128, NT], f16, tag="L", name=f"L{b}{k}")
            nc.scalar.activation(L[:], sig2[:], ACTF.Ln)
            # per-cap cost: smp[c8, t] = sum_p L ; bd8 entries are 0.5
            smp = pps_q.tile([128, 512], f32, tag="q", name=f"smp{b}{k}")
            nc.tensor.matmul(smp[0:8, 0:NT], bd8[:], L[:], start=True, stop=True)
            c1 = psm.tile([8, NT], f32, tag="c1", name=f"c1{b}{k}")
            nc.vector.tensor_tensor(c1[:], smp[0:8, 0:NT], bv8[:], op=ALU.add)
            c2 = psm.tile([8, NT], f32, tag="c2", name=f"c2{b}{k}")
            nc.vector.tensor_tensor(c2[:], c1[:], s08_ps[0:8, 0:NT], op=ALU.mult)
            wk = (1.0 / O_DIM) if k == 0 else 1.0
            ain = psm.tile([8, NT], f32, tag="ain", name=f"ain{b}{k}")
            nc.vector.scalar_tensor_tensor(
                ain[:], c2[:], -wk, ba8[:], op0=ALU.mult, op1=ALU.add)
            # a = sigmoid(LAMBDA * ain) = 1 / (1 + exp(-LAMBDA * ain))
            ea = psm.tile([8, NT], f32, tag="ea", name=f"ea{b}{k}")
            nc.scalar.activation(ea[:], ain[:], ACTF.Exp, scale=-LAMBDA)
            ua = psm.tile([8, NT], f32, tag="ua", name=f"ua{b}{k}")
            nc.vector.tensor_scalar(ua[:], ea[:], 1.0, None, op0=ALU.add)
            a8 = psm.tile([8, NT], f32, tag="a8", name=f"a8{b}{k}")
            nc.vector.reciprocal(a8[:], ua[:])

            if k == num_routing - 1:
                nc.sync.dma_start(out_mu[b].rearrange("t r -> r t"), mu[:])
                nc.sync.dma_start(out_a[b].rearrange("t c -> c t"), a8[:])
            else:
                a816 = psm.tile([8, NT], f16, tag="a816", name=f"a816{b}{k}")
                with nc.allow_low_precision(reason="a broadcast; tol 2e-2"):
                    nc.vector.tensor_scalar(a816[:], a8[:], 1.0, None,
                                            op0=ALU.mult)
                arep = pps_q.tile([128, 512], f32, tag="q", name=f"ar{b}{k}")
                nc.tensor.matmul(arep[:, 0:NT], bdt8[:], a816[:],
                                 start=True, stop=True)
                lnA = psm.tile([128, NT], f32, tag="lnA", name=f"lnA{b}{k}")
                nc.scalar.activation(lnA[:], arep[:, 0:NT], ACTF.Ln, bias=eps_col[:])
                # lnE = lnA - 0.5 ln(sig2) - 0.5 ln(2pi)
                lnE = psm.tile([128, NT], f32, tag="lnE", name=f"lnE{b}{k}")
                nc.vector.scalar_tensor_tensor(
                    lnE[:], L[:], -0.5, lnA[:], op0=ALU.mult, op1=ALU.add)
                nc.vector.tensor_scalar(lnE[:], lnE[:], -0.5 * LN_2PI, None,
                                        op0=ALU.add)
                rsig = psm.tile([128, NT], f32, tag="rsig", name=f"rv{b}{k}")
                nc.vector.reciprocal(rsig[:], sig2[:])
                negg = psm.tile([128, NT], f32, tag="negg", name=f"ng{b}{k}")
                nc.vector.tensor_scalar_mul(negg[:], rsig[:], -0.5)
                if N_ACT > 0:
                    # sqrt(g) = exp(0.5 ln(0.5 rsig)); bng = -sqrt(g) mu
                    lng = psm.tile([128, NT], f32, tag="lng", name=f"lg{b}{k}")
                    nc.scalar.activation(lng[:], rsig[:], ACTF.Ln, scale=0.5)
                    sqg = psm.tile([128, NT], f32, tag="sqg", name=f"sq{b}{k}")
                    nc.scalar.activation(sqg[:], lng[:], ACTF.Exp, scale=0.5)
                    bng = psm.tile([128, NT], f32, tag="bng", name=f"bg{b}{k}")
                    nc.vector.scalar_tensor_tensor(
                        bng[:], sqg[:], -1.0, mu[:], op0=ALU.mult, op1=ALU.mult)
                else:
                    sqg = bng = negg
                params[b] = (mu, negg, lnE, sqg, bng)

        # ---------------- schedule: batch pairs, AR hidden ----------------
        def drain(g):
            r = None
            while True:
                try:
                    r = next(g)
                except StopIteration as e:
                    return e.value

        for b0 in range(0, B, 2):
            b1 = b0 + 1
            load(b0)
            load(b1)
            load(b0 + 2)
            drain(p2_gen(b0, 0, ats[b0], avs[b0]))
            drain(p2_gen(b1, 0, ats[b1], avs[b1]))
            for k in range(1, num_routing):
                h0 = drain(p1_gen(b0, k))
                h1 = drain(p1_gen(b1, k))
                q0 = qmini(b0, k, *h0)
                drain(p2_gen(b0, k, *q0))
                q1 = qmini(b1, k, *h1)
                drain(p2_gen(b1, k, *q1))

    if split_waits:
        _split_sync_waits(nc)
    return nc


# ------------------------- host-side wrapper ----------------------------

def make_consts():
    """Selector/mask constants for the p-major (p, c8) packing."""
    # p-sum selectors: rows (8p + c8) of tile t -> cap col 8t + c8
    bdp = np.zeros((128, NTM, 128), np.float16)
    for t in range(NTM):
        for p in range(16):
            for c8 in range(8):
                bdp[8 * p + c8, t, 8 * t + c8] = 1.0
    bdov = np.zeros((128, 2, NOV), np.float16)
    for tv in range(2):
        for p in range(16):
            for c8 in range(8):
                bdov[8 * p + c8, tv, 8 * tv + c8] = 1.0
    # S0 spread: A[k, r] = 1 iff k % 8 == r % 8
    amat = np.zeros((128, 128), np.float16)
    for kk in range(128):
        for r in range(kk % 8, 128, 8):
            amat[kk, r] = 1.0
    # tile masks: M[k, t] = 1 iff k // 8 == t
    mmask = np.zeros((128, NT), np.float16)
    for kk in range(128):
        mmask[kk, kk // 8] = 1.0
    m2mask = np.zeros((NOV, 2), np.float16)
    for kk in range(NOV):
        m2mask[kk, kk // 8] = 1.0
    # p-reduce within cap, folded 0.5 for log sigma = 0.5 ln sig2
    bd8 = np.zeros((128, 8), np.float16)
    for p in range(16):
        for c8 in range(8):
            bd8[8 * p + c8, c8] = 0.5
    # a broadcast: [8, NT] -> (p, c8) rows
    bdt8 = np.zeros((8, 128), np.float16)
    for c8 in range(8):
        for p in range(16):
            bdt8[c8, 8 * p + c8] = 1.0
    return bdp, bdov, amat, mmask, m2mask, bd8, bdt8


def _get_nc():
    key = "full"
    if key not in _NC_CACHE:
        _NC_CACHE[key] = build_nc()
    return _NC_CACHE[key]


def make_in_maps(votes, beta_v, beta_a):
    """votes [B, I, C, D] f32 -> per-core input dicts (p-major packing)."""
    bvc = 16.0 * np.repeat(beta_v.reshape(-1), WW)   # [C], pre-scaled by P
    bac = np.repeat(beta_a.reshape(-1), WW)
    bdp_np, bdov_np, am_np, mm_np, m2_np, bd8_np, bdt8_np = make_consts()
    vt_all = np.ascontiguousarray(votes.transpose(0, 2, 3, 1))  # [B, C, D, I]
    in_maps = []
    for c in range(N_CORES):
        sl = slice(c * CL, (c + 1) * CL)
        blk = vt_all[:, sl]                               # [B, CL, D, I]
        pose = blk[:, :, :P_DIM, :].astype(np.float16)    # [B, CL, 16, I]
        # [B, CL=18*8 caps, 16 pose, I] -> [B, t, p, c8, i] -> [B,t,8p+c8,i]
        vt = np.ascontiguousarray(
            pose.reshape(B, NT, 8, P_DIM, I_DIM).transpose(0, 1, 3, 2, 4)
            .reshape(B, NT, 128, I_DIM))
        acts = blk[:, :, P_DIM, :].astype(np.float16)     # [B, CL, I]
        at = np.ascontiguousarray(acts[:, :128, :])
        av = np.ascontiguousarray(acts[:, 128:, :])
        # per-cap consts in [c8, t] layout: col t, row c8 -> cap 8t + c8
        cl_idx = np.arange(CL)
        bv8 = np.ascontiguousarray(
            bvc[c * CL + cl_idx].reshape(NT, 8).T.astype(np.float32))
        ba8 = np.ascontiguousarray(
            bac[c * CL + cl_idx].reshape(NT, 8).T.astype(np.float32))
        in_maps.append({
            "vt": vt, "at": at, "av": av,
            "bdp": np.ascontiguousarray(bdp_np.reshape(128, NTM * 128)),
            "bdov": np.ascontiguousarray(bdov_np.reshape(128, 2 * NOV)),
            "am": am_np, "mm": mm_np, "m2": m2_np,
            "bd8": bd8_np, "bdt8": bdt8_np,
            "bv8": bv8, "ba8": ba8,
        })
    return in_maps


def assemble_output(results):
    """Per-core out_mu [B, NT, 128] + out_a [B, NT, 8] -> [B, O, w, w, D].

    out_mu rows are p-major: row (8p + c8) of tile t = cap 8t+c8, pose p.
    """
    full = np.zeros((B, C_DIM, D_DIM), np.float32)
    for c in range(N_CORES):
        om = np.asarray(results[c]["out_mu"])             # [B, NT, 128]
        oa = np.asarray(results[c]["out_a"])              # [B, NT, 8]
        sl = slice(c * CL, (c + 1) * CL)
        # [B, t, (p, c8)] -> [B, t, c8, p] -> [B, CL, P]
        mu = om.reshape(B, NT, P_DIM, 8).transpose(0, 1, 3, 2)
        full[:, sl, :P_DIM] = mu.reshape(B, CL, P_DIM)
        full[:, sl, P_DIM] = oa.reshape(B, CL)
    w = int(math.sqrt(C_DIM // O_DIM))
    return full.reshape(B, O_DIM, w, w, D_DIM).astype(np.float32)


def kernel(**inputs) -> np.ndarray:
    from concourse.bass_utils import run_bass_kernel_spmd

    votes = np.ascontiguousarray(np.asarray(inputs["votes"], dtype=np.float32))
    beta_v = np.asarray(inputs["beta_v"], dtype=np.float32)
    beta_a = np.asarray(inputs["beta_a"], dtype=np.float32)
    output_dim = int(np.asarray(inputs["output_dim"]))
    num_routing = int(np.asarray(inputs["num_routing"]))
    assert votes.shape == (B, I_DIM, C_DIM, D_DIM), votes.shape
    assert output_dim == O_DIM and num_routing == NUM_ROUTING

    nc = _get_nc()
    in_maps = make_in_maps(votes, beta_v, beta_a)
    res = run_bass_kernel_spmd(nc, in_maps, list(range(N_CORES)))
    return assemble_output([res.results[i] for i in range(N_CORES)])


# revision 50
# speedup vs baseline: 1.3072x; 1.0885x over previous
"""Trainium2 Bass kernel for EM matrix-capsule routing (nn_MatrixRouting).

Problem shapes (hardcoded): votes [4, 1152, 1152, 17] f32, beta_v [1,32,1,1],
beta_a [1,32,1], output_dim=32, num_routing=3. Output [4, 32, 6, 6, 17].

Strategy: shard the output-capsule axis C=1152 across 8 cores (144 each).
Host pre-transposes each core's vote shard to a p-major (p,c8)-on-partition
fp16 layout: 18 tiles of [128 = 16 pose x 8 caps, I=1152] per batch; the
shard stays SBUF-resident across all 3 EM iterations.

Per-partition EM params (mu, -g, lnE) make the Gaussian 1-3 ops; p-sums and
row-sums are tiny shared-selector matmuls on TensorE; the q -> (c,p) row
replication is a single stride-0-partition SBUF->SBUF DMA per tile (p-major
makes the replicated view contiguous); stats products run as 2x-mode
tensor_tensor with 4x-mode tensor_scalar accumulates (a few tiles on
GpSimd for balance). One activation-table set (exp/ln/square/identity)
serves the whole kernel: sqrt -> 0.5*ln, sigmoid -> exp + tiny reciprocal.
The only cross-core exchange is the [1, I] R-normalizer row, AllReduce'd
per (batch, iteration) and hidden under the paired batch's compute.
"""

import math
import numpy as np
from contextlib import ExitStack

# ---- problem constants (hardcoded per the task contract) ----
B = 4
I_DIM = 1152
C_DIM = 1152
P_DIM = 16
D_DIM = 17
N_CORES = 8
NUM_ROUTING = 3
O_DIM = 32
WW = 36  # w*w = 6*6 positions per output capsule

CL = C_DIM // N_CORES        # 144 local caps
NT = CL * P_DIM // 128       # 18 tiles of [128, I]
NTM = 16                     # tiles whose caps fit the main [128, I] c-pack
NOV = CL - 128               # 16 overflow caps (tiles 16, 17)
NS = 3                       # i-slices per tile for PSUM bank alignment
SW = I_DIM // NS             # 384 columns per slice

EPS = 1e-10
LAMBDA = 1e-4
LN_2PI = math.log(2.0 * math.pi)

# ---- tuning knobs ----
N_ACT = 18                   # tiles using the scalar-engine Square form of p1
GP_TILES = ()           # p2 tiles whose products run on GpSimd

_NC_CACHE = {}


def _patch_tile_drain():
    """This walrus build only accepts one sync-wait on a CTRL instruction;
    spread the Tile exit-drain waits across single-wait NOPs."""
    import concourse.tile as tile
    import concourse.mybir as mybir
    from concourse.vector_clock import ScopedClock

    if getattr(tile.TileContext, "_drain_patched", False):
        return

    def _patched(self, tick_clock, wait_clock):
        nc = self.nc
        probe = nc.sync.nop()
        wait_clock.add_sem_waits(
            probe.ins, ScopedClock({None: tick_clock.global_clock})
        )
        waits = list(probe.ins.sync_info.on_wait) if probe.ins.sync_info else []
        if probe.ins.sync_info:
            probe.ins.sync_info.on_wait = waits[:1]
        for w in waits[1:]:
            n2 = nc.sync.nop()
            if n2.ins.sync_info is None:
                n2.ins.sync_info = mybir.SyncInfo(on_wait=[w], on_update=[])
            else:
                n2.ins.sync_info.on_wait = [w]
        nc.sync.drain()
        nc.all_engine_barrier()
        assert self.sems is not None
        popped = nc._tile_sem_poison_stack.pop()
        assert popped is self._sem_poison
        nc.clear_and_free_semaphores(list(self.sems.allocated().values()))
        nc.all_engine_barrier()

    tile.TileContext._drain_and_barrier = _patched
    tile.TileContext._drain_patched = True


def _split_sync_waits(nc, max_waits=1):
    """This walrus build accepts at most one sync-wait per instruction;
    move excess waits onto preceding same-engine NOPs."""
    import concourse.mybir as mybir

    uid = [0]
    for fn in nc.m.functions:
        for bb in fn.blocks:
            insts = bb.instructions
            out = []
            for inst in insts:
                si = inst.sync_info
                if si is not None and si.on_wait and len(si.on_wait) > max_waits:
                    waits = list(si.on_wait)
                    keep = waits[-max_waits:]
                    for w in waits[:-max_waits]:
                        uid[0] += 1
                        nop = mybir.InstNoOp(
                            name=f"I-waitsplit-{uid[0]}", ins=[], outs=[])
                        nop.engine = inst.engine
                        nop.sync_info = mybir.SyncInfo(on_wait=[w], on_update=[])
                        out.append(nop)
                    si.on_wait = keep
                out.append(inst)
            bb.instructions = out
    return nc


def build_nc(num_routing=NUM_ROUTING, split_waits=True):
    """Build the per-core SPMD Bass program (identical on every core)."""
    import concourse.bass as bass
    import concourse.mybir as mybir
    import concourse.tile as tile

    _patch_tile_drain()

    f32 = mybir.dt.float32
    f16 = mybir.dt.float16
    ALU = mybir.AluOpType
    ACTF = mybir.ActivationFunctionType

    nc = bass.Bass()
    vt_in = nc.declare_dram_parameter("vt", [B, NT, 128, I_DIM], f16, isOutput=False)
    at_in = nc.declare_dram_parameter("at", [B, 128, I_DIM], f16, isOutput=False)
    av_in = nc.declare_dram_parameter("av", [B, NOV, I_DIM], f16, isOutput=False)
    bdp_in = nc.declare_dram_parameter("bdp", [128, NTM * 128], f16, isOutput=False)
    bdov_in = nc.declare_dram_parameter("bdov", [128, 2 * NOV], f16, isOutput=False)
    am_in = nc.declare_dram_parameter("am", [128, 128], f16, isOutput=False)
    mm_in = nc.declare_dram_parameter("mm", [128, NT], f16, isOutput=False)
    m2_in = nc.declare_dram_parameter("m2", [NOV, 2], f16, isOutput=False)
    bd8_in = nc.declare_dram_parameter("bd8", [128, 8], f16, isOutput=False)
    bdt8_in = nc.declare_dram_parameter("bdt8", [8, 128], f16, isOutput=False)
    bv8_in = nc.declare_dram_parameter("bv8", [8, NT], f32, isOutput=False)
    ba8_in = nc.declare_dram_parameter("ba8", [8, NT], f32, isOutput=False)
    out_mu = nc.declare_dram_parameter("out_mu", [B, NT, 128], f32, isOutput=True)
    out_a = nc.declare_dram_parameter("out_a", [B, NT, 8], f32, isOutput=True)
    rs_loc = nc.dram_tensor("rs_loc", [B, I_DIM], f32)
    rs_sh = nc.dram_tensor("rs_sh", [B, I_DIM], f32, addr_space="Shared")
    qd = nc.dram_tensor("qd", [B, CL, I_DIM], f16)
    rcpd = nc.dram_tensor("rcpd", [B, 1, I_DIM], f16)

    groups = [list(range(N_CORES))]

    with tile.TileContext(nc) as tc, ExitStack() as ctx:
        pconst = ctx.enter_context(tc.tile_pool(name="const", bufs=1))
        pv = ctx.enter_context(tc.tile_pool(name="vt", bufs=2))
        pat = ctx.enter_context(tc.tile_pool(name="at", bufs=3))
        pwk = ctx.enter_context(tc.tile_pool(name="wk", bufs=2))
        pe_ = ctx.enter_context(tc.tile_pool(name="ex", bufs=2))
        pap = ctx.enter_context(tc.tile_pool(name="apS", bufs=2))
        pqc = ctx.enter_context(tc.tile_pool(name="qc", bufs=3))
        ps1 = ctx.enter_context(tc.tile_pool(name="s1o", bufs=2))
        prs = ctx.enter_context(tc.tile_pool(name="rs", bufs=2))
        psm = ctx.enter_context(tc.tile_pool(name="sm", bufs=2))
        # PSUM budget (8 banks): ap ring 3 (shared with rp) + ov 3 + q ring 2
        pps_ap = ctx.enter_context(tc.tile_pool(name="pap", bufs=1, space="PSUM"))
        pps_ov = ctx.enter_context(tc.tile_pool(name="pov", bufs=1, space="PSUM"))
        pps_q = ctx.enter_context(tc.tile_pool(name="pq", bufs=2, space="PSUM"))

        # ---- constants ----
        bdp = pconst.tile([128, NTM, 128], f16)
        nc.sync.dma_start(bdp[:].rearrange("p a b -> p (a b)"), bdp_in[:])
        bdov = pconst.tile([128, 2, NOV], f16)
        nc.sync.dma_start(bdov[:].rearrange("p a b -> p (a b)"), bdov_in[:])
        amat = pconst.tile([128, 128], f16)
        nc.sync.dma_start(amat[:], am_in[:])
        mmask = pconst.tile([128, NT], f16)
        nc.sync.dma_start(mmask[:], mm_in[:])
        m2mask = pconst.tile([NOV, 2], f16)
        nc.sync.dma_start(m2mask[:], m2_in[:])
        bd8 = pconst.tile([128, 8], f16)
        nc.sync.dma_start(bd8[:], bd8_in[:])
        bdt8 = pconst.tile([8, 128], f16)
        nc.sync.dma_start(bdt8[:], bdt8_in[:])
        bv8 = pconst.tile([8, NT], f32)
        nc.sync.dma_start(bv8[:], bv8_in[:])
        ba8 = pconst.tile([8, NT], f32)
        nc.sync.dma_start(ba8[:], ba8_in[:])
        onesA = pconst.tile([128, 1], f16)
        nc.vector.memset(onesA[:], 1.0)
        onesV = pconst.tile([NOV, 1], f16)
        nc.vector.memset(onesV[:], 1.0)
        eps_col = pconst.tile([128, 1], f32)
        nc.vector.memset(eps_col[:], EPS)

        vts, ats, avs = {}, {}, {}
        loaded = set()

        def load(b):
            if b >= B or b in loaded:
                return
            loaded.add(b)
            vt = pv.tile([128, NT, I_DIM], f16, tag="vt", name=f"vt{b}")
            nc.sync.dma_start(
                vt[:], vt_in[b].rearrange("t p i -> p t i"))
            at = pat.tile([128, I_DIM], f16, tag="at", name=f"at{b}")
            nc.sync.dma_start(at[:], at_in[b])
            av = pat.tile([NOV, I_DIM], f16, tag="av", name=f"av{b}")
            nc.sync.dma_start(av[:], av_in[b])
            vts[b], ats[b], avs[b] = vt, at, av

        params = {}   # b -> (mu, negg, lnE, sqg, bng) [128, NT] f32 tiles

        def p1_gen(b, k):
            """Gaussian weights e -> per-cap p-sum ap (f16 SBUF), local
            row-sum, AllReduce launch. Yields once per tile."""
            vt = vts[b]
            mu_t, negg_t, lnE_t, sqg_t, bng_t = params[b]
            ap_ps = pps_ap.tile([128, NS, 512], f32, tag="ap", name=f"ap{b}_{k}")
            ov_ps = pps_ov.tile([NOV, NS, 512], f32, tag="ov", name=f"ov{b}_{k}")
            for t in range(NT):
                V = vt[:, t, :]
                e = pe_.tile([128, I_DIM], f16, tag="e", name=f"e{b}{k}{t}")
                if t < N_ACT:
                    # ACT form: u = (sqrt(g) V - sqrt(g) mu)^2 ; e = exp(-u+lnE)
                    u = pwk.tile([128, I_DIM], f16, tag="d", name=f"u{b}{k}{t}")
                    nc.scalar.activation(u[:], V, ACTF.Square,
                                         bias=bng_t[:, t : t + 1],
                                         scale=sqg_t[:, t : t + 1])
                    nc.scalar.activation(e[:], u[:], ACTF.Exp,
                                         bias=lnE_t[:, t : t + 1], scale=-1.0)
                else:
                    # DVE form: d2 = (V - mu)^2 ; e = exp(negg d2 + lnE)
                    d = pwk.tile([128, I_DIM], f16, tag="d", name=f"d{b}{k}{t}")
                    nc.vector.tensor_scalar(
                        d[:], V, mu_t[:, t : t + 1], None, op0=ALU.subtract)
                    d2 = pwk.tile([128, I_DIM], f16, tag="z", name=f"d2{b}{k}{t}")
                    nc.vector.tensor_tensor(d2[:], d[:], d[:], op=ALU.mult)
                    nc.scalar.activation(e[:], d2[:], ACTF.Exp,
                                         bias=lnE_t[:, t : t + 1],
                                         scale=negg_t[:, t : t + 1])
                # p-sum: rows (p, c8) of tile t -> cap row 8t + c8
                for s in range(NS):
                    rhs = e[:, s * SW : (s + 1) * SW]
                    if t < NTM:
                        nc.tensor.matmul(ap_ps[:, s, 0:SW], bdp[:, t, :], rhs,
                                         start=(t == 0), stop=(t == NTM - 1))
                    else:
                        nc.tensor.matmul(ov_ps[:, s, 0:SW], bdov[:, t - NTM, :],
                                         rhs, start=(t == NTM), stop=(t == NT - 1))
                yield
            # free PSUM early: f16 copies of the cap-packed ap
            apS = pap.tile([128, I_DIM], f16, tag="apS", name=f"apS{b}{k}")
            nc.scalar.activation(
                apS[:].rearrange("p (s x) -> p s x", s=NS),
                ap_ps[:, :, 0:SW], ACTF.Identity)
            ovS = pap.tile([NOV, I_DIM], f16, tag="ovS", name=f"ovS{b}{k}")
            nc.scalar.activation(
                ovS[:].rearrange("p (s x) -> p s x", s=NS),
                ov_ps[:, :, 0:SW], ACTF.Identity)
            # local row-sum over all c: ones-matmuls on the f16 ap copies
            # (reuses the ap psum banks, which are dead after the copies)
            rp = pps_ap.tile([128, NS, 512], f32, tag="ap", name=f"rp{b}{k}")
            for s in range(NS):
                nc.tensor.matmul(rp[0:1, s, 0:SW], onesA[:],
                                 apS[:, s * SW : (s + 1) * SW],
                                 start=True, stop=False)
                nc.tensor.matmul(rp[0:1, s, 0:SW], onesV[:],
                                 ovS[:, s * SW : (s + 1) * SW],
                                 start=False, stop=True)
            rs_row = prs.tile([1, I_DIM], f32, tag="rsrow", name=f"rsrow{b}{k}")
            nc.scalar.activation(rs_row[:].rearrange("p (s x) -> p s x", s=NS),
                                 rp[0:1, :, 0:SW], ACTF.Identity)
            nc.sync.dma_start(rs_loc[b], rs_row[:])
            nc.gpsimd.collective_compute(
                "AllReduce", ALU.add, replica_groups=groups,
                ins=[rs_loc[b]], outs=[rs_sh[b]])
            return apS, ovS

        def qmini(b, k, apS, ovS):
            """R-normalize + vote-activation weight -> cap-packed q in SBUF."""
            # read the AllReduce'd row as [128, 9] so the iterative
            # reciprocal runs partition-parallel (~60 cyc, not ~7.5us)
            rsg = prs.tile([128, 9], f32, tag="rsg", name=f"rsg{b}{k}")
            nc.sync.dma_start(rsg[:], rs_sh[b].rearrange("(r j) -> r j", j=9))
            nc.vector.tensor_scalar(rsg[:], rsg[:], EPS, None, op0=ALU.add)
            rcpf = prs.tile([128, 9], f32, tag="rcpf", name=f"rcpf{b}{k}")
            nc.vector.reciprocal(rcpf[:], rsg[:])
            rcp1 = prs.tile([128, 9], f16, tag="rcp1", name=f"rcp1{b}{k}")
            with nc.allow_low_precision(
                    reason="f16 R-normalizer; gate tolerance 2e-2"):
                nc.vector.tensor_scalar(rcp1[:], rcpf[:], 1.0, None,
                                        op0=ALU.mult)
            # broadcast the row to all partitions via a DRAM bounce
            nc.sync.dma_start(
                rcpd[b].rearrange("o (r j) -> (o r) j", j=9), rcp1[:])
            rcp = prs.tile([128, I_DIM], f16, tag="rcp", name=f"rcp{b}{k}")
            nc.sync.dma_start(rcp[:], rcpd[b].broadcast_to((128, I_DIM)))
            qp = ps1.tile([128, I_DIM], f16, tag="qp", name=f"qp{b}{k}")
            nc.vector.tensor_tensor(qp[:], apS[:], rcp[:], op=ALU.mult)
            nc.vector.tensor_tensor(qp[:], qp[:], ats[b][:], op=ALU.mult)
            qv = ps1.tile([NOV, I_DIM], f16, tag="qv", name=f"qv{b}{k}")
            nc.vector.tensor_tensor(qv[:], ovS[:], rcp[0:NOV, :], op=ALU.mult)
            nc.vector.tensor_tensor(qv[:], qv[:], avs[b][:], op=ALU.mult)
            nc.sync.dma_start(qd[b, 0:128], qp[:])
            nc.sync.dma_start(qd[b, 128:CL], qv[:])
            return qp, qv

        def p2_gen(b, k, qp, qv):
            """Stats via DMA-broadcast q + 2x products + 4x accumulates,
            then the small per-cap math."""
            vt = vts[b]
            S1 = psm.tile([128, NT], f32, tag="S1", name=f"S1{b}{k}")
            S2 = psm.tile([128, NT], f32, tag="S2", name=f"S2{b}{k}")
            # S0 (= sum_i q) per cap via in-place identity + accum, then
            # selector-matmuls spread it to (p,c8) rows and [8, NT] form
            S0q = psm.tile([128, 1], f32, tag="S0q", name=f"S0q{b}{k}")
            nc.vector.tensor_scalar(qp[:], qp[:], 1.0, 0.0, op0=ALU.mult,
                                    op1=ALU.add, accum_out=S0q[:])
            S0v = psm.tile([NOV, 1], f32, tag="S0v", name=f"S0v{b}{k}")
            nc.vector.tensor_scalar(qv[:], qv[:], 1.0, 0.0, op0=ALU.mult,
                                    op1=ALU.add, accum_out=S0v[:])
            Bm = psm.tile([128, NT], f16, tag="Bm", name=f"Bm{b}{k}")
            nc.vector.tensor_scalar(Bm[:], mmask[:], S0q[:], None, op0=ALU.mult)
            B2 = psm.tile([NOV, 2], f16, tag="B2", name=f"B2{b}{k}")
            nc.vector.tensor_scalar(B2[:], m2mask[:], S0v[:], None, op0=ALU.mult)
            s0w_ps = pps_q.tile([128, 512], f32, tag="q", name=f"s0w{b}{k}")
            nc.tensor.matmul(s0w_ps[:, 0:NTM], amat[:], Bm[:, 0:NTM],
                             start=True, stop=True)
            nc.tensor.matmul(s0w_ps[:, NTM:NT], amat[0:NOV, :], B2[:],
                             start=True, stop=True)
            # S0 arranged [8, NT] for the per-cap cost math
            s08_ps = pps_q.tile([128, 512], f32, tag="q", name=f"s08{b}{k}")
            nc.tensor.matmul(s08_ps[0:8, 0:NTM], amat[:, 0:8], Bm[:, 0:NTM],
                             start=True, stop=True)
            nc.tensor.matmul(s08_ps[0:8, NTM:NT], amat[0:NOV, 0:8], B2[:],
                             start=True, stop=True)

            for t in range(NT):
                V = vt[:, t, :]
                qcp = pqc.tile([128, I_DIM], f16, tag="qc", name=f"qc{b}{k}{t}")
                if k == 0:
                    if t < NTM:
                        src = at_in[b, 8 * t : 8 * t + 8, :]
                    else:
                        src = av_in[b, 8 * (t - NTM) : 8 * (t - NTM) + 8, :]
                else:
                    if t < NTM:
                        src = qd[b, 8 * t : 8 * t + 8, :]
                    else:
                        src = qd[b, 128 + 8 * (t - NTM) :
                                 128 + 8 * (t - NTM) + 8, :]
                nc.sync.dma_start(qcp[:], src.partition_broadcast(16))
                s1o = ps1.tile([128, I_DIM], f16, tag="s1o", name=f"s1o{b}{k}{t}")
                s2o = ps1.tile([128, I_DIM], f16, tag="s2o", name=f"s2o{b}{k}{t}")
                # DVE accumulation is 1x-only, so the fused STT (product +
                # accum in one pass) is optimal; spill some tiles' s1o to
                # GpSimd product + scalar-engine accumulate for balance.
                if t in GP_TILES:
                    nc.gpsimd.tensor_tensor(s1o[:], qcp[:], V, op=ALU.mult)
                    nc.scalar.activation(s1o[:], s1o[:], ACTF.Identity,
                                         accum_out=S1[:, t : t + 1])
                else:
                    nc.vector.scalar_tensor_tensor(
                        s1o[:], qcp[:], 1.0, V, op0=ALU.mult, op1=ALU.mult,
                        accum_out=S1[:, t : t + 1])
                nc.vector.scalar_tensor_tensor(
                    s2o[:], s1o[:], 1.0, V, op0=ALU.mult, op1=ALU.mult,
                    accum_out=S2[:, t : t + 1])
                yield

            # ---- small math on [128, NT] f32 (p-major rows) ----
            rS = psm.tile([128, NT], f32, tag="rS", name=f"rS{b}{k}")
            nc.vector.reciprocal(rS[:], s0w_ps[:, 0:NT])
            mu = psm.tile([128, NT], f32, tag="mu", name=f"mu{b}{k}")
            nc.vector.tensor_tensor(mu[:], S1[:], rS[:], op=ALU.mult)
            ex2 = psm.tile([128, NT], f32, tag="ex2", name=f"ex2{b}{k}")
            nc.vector.tensor_tensor(ex2[:], S2[:], rS[:], op=ALU.mult)
            mu2 = psm.tile([128, NT], f32, tag="mu2", name=f"mu2{b}{k}")
            nc.vector.tensor_tensor(mu2[:], mu[:], mu[:], op=ALU.mult)
            sig2 = psm.tile([128, NT], f32, tag="sig2", name=f"sig2{b}{k}")
            nc.vector.tensor_tensor(sig2[:], ex2[:], mu2[:], op=ALU.subtract)
            nc.vector.tensor_scalar_max(sig2[:], sig2[:], 1e-12)
            # log sigma = 0.5 ln(sig2); the 0.5 is folded into bd8/lnE uses
            L = psm.tile([128, NT], f16, tag="L", name=f"L{b}{k}")
            nc.scalar.activation(L[:], sig2[:], ACTF.Ln)
            # per-cap cost: smp[c8, t] = sum_p L ; bd8 entries are 0.5
            smp = pps_q.tile([128, 512], f32, tag="q", name=f"smp{b}{k}")
            nc.tensor.matmul(smp[0:8, 0:NT], bd8[:], L[:], start=True, stop=True)
            c1 = psm.tile([8, NT], f32, tag="c1", name=f"c1{b}{k}")
            nc.vector.tensor_tensor(c1[:], smp[0:8, 0:NT], bv8[:], op=ALU.add)
            c2 = psm.tile([8, NT], f32, tag="c2", name=f"c2{b}{k}")
            nc.vector.tensor_tensor(c2[:], c1[:], s08_ps[0:8, 0:NT], op=ALU.mult)
            wk = (1.0 / O_DIM) if k == 0 else 1.0
            ain = psm.tile([8, NT], f32, tag="ain", name=f"ain{b}{k}")
            nc.vector.scalar_tensor_tensor(
                ain[:], c2[:], -wk, ba8[:], op0=ALU.mult, op1=ALU.add)
            # a = sigmoid(LAMBDA * ain) = 1 / (1 + exp(-LAMBDA * ain))
            ea = psm.tile([8, NT], f32, tag="ea", name=f"ea{b}{k}")
            nc.scalar.activation(ea[:], ain[:], ACTF.Exp, scale=-LAMBDA)
            ua = psm.tile([8, NT], f32, tag="ua", name=f"ua{b}{k}")
            nc.vector.tensor_scalar(ua[:], ea[:], 1.0, None, op0=ALU.add)
            a8 = psm.tile([8, NT], f32, tag="a8", name=f"a8{b}{k}")
            nc.vector.reciprocal(a8[:], ua[:])

            if k == num_routing - 1:
                nc.sync.dma_start(out_mu[b].rearrange("t r -> r t"), mu[:])
                nc.sync.dma_start(out_a[b].rearrange("t c -> c t"), a8[:])
            else:
                a816 = psm.tile([8, NT], f16, tag="a816", name=f"a816{b}{k}")
                with nc.allow_low_precision(reason="a broadcast; tol 2e-2"):
                    nc.vector.tensor_scalar(a816[:], a8[:], 1.0, None,
                                            op0=ALU.mult)
                arep = pps_q.tile([128, 512], f32, tag="q", name=f"ar{b}{k}")
                nc.tensor.matmul(arep[:, 0:NT], bdt8[:], a816[:],
                                 start=True, stop=True)
                lnA = psm.tile([128, NT], f32, tag="lnA", name=f"lnA{b}{k}")
                nc.scalar.activation(lnA[:], arep[:, 0:NT], ACTF.Ln, bias=eps_col[:])
                # lnE = lnA - 0.5 ln(sig2) - 0.5 ln(2pi)
                lnE = psm.tile([128, NT], f32, tag="lnE", name=f"lnE{b}{k}")
                nc.vector.scalar_tensor_tensor(
                    lnE[:], L[:], -0.5, lnA[:], op0=ALU.mult, op1=ALU.add)
                nc.vector.tensor_scalar(lnE[:], lnE[:], -0.5 * LN_2PI, None,
                                        op0=ALU.add)
                rsig = psm.tile([128, NT], f32, tag="rsig", name=f"rv{b}{k}")
                nc.vector.reciprocal(rsig[:], sig2[:])
                negg = psm.tile([128, NT], f32, tag="negg", name=f"ng{b}{k}")
                nc.vector.tensor_scalar_mul(negg[:], rsig[:], -0.5)
                if N_ACT > 0:
                    # sqrt(g) = exp(0.5 ln(0.5 rsig)); bng = -sqrt(g) mu
                    lng = psm.tile([128, NT], f32, tag="lng", name=f"lg{b}{k}")
                    nc.scalar.activation(lng[:], rsig[:], ACTF.Ln, scale=0.5)
                    sqg = psm.tile([128, NT], f32, tag="sqg", name=f"sq{b}{k}")
                    nc.scalar.activation(sqg[:], lng[:], ACTF.Exp, scale=0.5)
                    bng = psm.tile([128, NT], f32, tag="bng", name=f"bg{b}{k}")
                    nc.vector.scalar_tensor_tensor(
                        bng[:], sqg[:], -1.0, mu[:], op0=ALU.mult, op1=ALU.mult)
                else:
                    sqg = bng = negg
                params[b] = (mu, negg, lnE, sqg, bng)

        # ---------------- schedule: batch pairs, AR hidden ----------------
        def drain(g):
            r = None
            while True:
                try:
                    r = next(g)
                except StopIteration as e:
                    return e.value

        for b0 in range(0, B, 2):
            b1 = b0 + 1
            load(b0)
            load(b1)
            load(b0 + 2)
            drain(p2_gen(b0, 0, ats[b0], avs[b0]))
            drain(p2_gen(b1, 0, ats[b1], avs[b1]))
            for k in range(1, num_routing):
                h0 = drain(p1_gen(b0, k))
                h1 = drain(p1_gen(b1, k))
                q0 = qmini(b0, k, *h0)
                drain(p2_gen(b0, k, *q0))
                q1 = qmini(b1, k, *h1)
                drain(p2_gen(b1, k, *q1))

    if split_waits:
        _split_sync_waits(nc)
    return nc


# ------------------------- host-side wrapper ----------------------------

def make_consts():
    """Selector/mask constants for the p-major (p, c8) packing."""
    # p-sum selectors: rows (8p + c8) of tile t -> cap col 8t + c8
    bdp = np.zeros((128, NTM, 128), np.float16)
    for t in range(NTM):
        for p in range(16):
            for c8 in range(8):
                bdp[8 * p + c8, t, 8 * t + c8] = 1.0
    bdov = np.zeros((128, 2, NOV), np.float16)
    for tv in range(2):
        for p in range(16):
            for c8 in range(8):
                bdov[8 * p + c8, tv, 8 * tv + c8] = 1.0
    # S0 spread: A[k, r] = 1 iff k % 8 == r % 8
    amat = np.zeros((128, 128), np.float16)
    for kk in range(128):
        for r in range(kk % 8, 128, 8):
            amat[kk, r] = 1.0
    # tile masks: M[k, t] = 1 iff k // 8 == t
    mmask = np.zeros((128, NT), np.float16)
    for kk in range(128):
        mmask[kk, kk // 8] = 1.0
    m2mask = np.zeros((NOV, 2), np.float16)
    for kk in range(NOV):
        m2mask[kk, kk // 8] = 1.0
    # p-reduce within cap, folded 0.5 for log sigma = 0.5 ln sig2
    bd8 = np.zeros((128, 8), np.float16)
    for p in range(16):
        for c8 in range(8):
            bd8[8 * p + c8, c8] = 0.5
    # a broadcast: [8, NT] -> (p, c8) rows
    bdt8 = np.zeros((8, 128), np.float16)
    for c8 in range(8):
        for p in range(16):
            bdt8[c8, 8 * p + c8] = 1.0
    return bdp, bdov, amat, mmask, m2mask, bd8, bdt8


def _get_nc():
    key = "full"
    if key not in _NC_CACHE:
        _NC_CACHE[key] = build_nc()
    return _NC_CACHE[key]


def make_in_maps(votes, beta_v, beta_a):
    """votes [B, I, C, D] f32 -> per-core input dicts (p-major packing)."""
    bvc = 16.0 * np.repeat(beta_v.reshape(-1), WW)   # [C], pre-scaled by P
    bac = np.repeat(beta_a.reshape(-1), WW)
    bdp_np, bdov_np, am_np, mm_np, m2_np, bd8_np, bdt8_np = make_consts()
    vt_all = np.ascontiguousarray(votes.transpose(0, 2, 3, 1))  # [B, C, D, I]
    in_maps = []
    for c in range(N_CORES):
        sl = slice(c * CL, (c + 1) * CL)
        blk = vt_all[:, sl]                               # [B, CL, D, I]
        pose = blk[:, :, :P_DIM, :].astype(np.float16)    # [B, CL, 16, I]
        # [B, CL=18*8 caps, 16 pose, I] -> [B, t, p, c8, i] -> [B,t,8p+c8,i]
        vt = np.ascontiguousarray(
            pose.reshape(B, NT, 8, P_DIM, I_DIM).transpose(0, 1, 3, 2, 4)
            .reshape(B, NT, 128, I_DIM))
        acts = blk[:, :, P_DIM, :].astype(np.float16)     # [B, CL, I]
        at = np.ascontiguousarray(acts[:, :128, :])
        av = np.ascontiguousarray(acts[:, 128:, :])
        # per-cap consts in [c8, t] layout: col t, row c8 -> cap 8t + c8
        cl_idx = np.arange(CL)
        bv8 = np.ascontiguousarray(
            bvc[c * CL + cl_idx].reshape(NT, 8).T.astype(np.float32))
        ba8 = np.ascontiguousarray(
            bac[c * CL + cl_idx].reshape(NT, 8).T.astype(np.float32))
        in_maps.append({
            "vt": vt, "at": at, "av": av,
            "bdp": np.ascontiguousarray(bdp_np.reshape(128, NTM * 128)),
            "bdov": np.ascontiguousarray(bdov_np.reshape(128, 2 * NOV)),
            "am": am_np, "mm": mm_np, "m2": m2_np,
            "bd8": bd8_np, "bdt8": bdt8_np,
            "bv8": bv8, "ba8": ba8,
        })
    return in_maps


def assemble_output(results):
    """Per-core out_mu [B, NT, 128] + out_a [B, NT, 8] -> [B, O, w, w, D].

    out_mu rows are p-major: row (8p + c8) of tile t = cap 8t+c8, pose p.
    """
    full = np.zeros((B, C_DIM, D_DIM), np.float32)
    for c in range(N_CORES):
        om = np.asarray(results[c]["out_mu"])             # [B, NT, 128]
        oa = np.asarray(results[c]["out_a"])              # [B, NT, 8]
        sl = slice(c * CL, (c + 1) * CL)
        # [B, t, (p, c8)] -> [B, t, c8, p] -> [B, CL, P]
        mu = om.reshape(B, NT, P_DIM, 8).transpose(0, 1, 3, 2)
        full[:, sl, :P_DIM] = mu.reshape(B, CL, P_DIM)
        full[:, sl, P_DIM] = oa.reshape(B, CL)
    w = int(math.sqrt(C_DIM // O_DIM))
    return full.reshape(B, O_DIM, w, w, D_DIM).astype(np.float32)


def kernel(**inputs) -> np.ndarray:
    from concourse.bass_utils import run_bass_kernel_spmd

    votes = np.ascontiguousarray(np.asarray(inputs["votes"], dtype=np.float32))
    beta_v = np.asarray(inputs["beta_v"], dtype=np.float32)
    beta_a = np.asarray(inputs["beta_a"], dtype=np.float32)
    output_dim = int(np.asarray(inputs["output_dim"]))
    num_routing = int(np.asarray(inputs["num_routing"]))
    assert votes.shape == (B, I_DIM, C_DIM, D_DIM), votes.shape
    assert output_dim == O_DIM and num_routing == NUM_ROUTING

    nc = _get_nc()
    in_maps = make_in_maps(votes, beta_v, beta_a)
    res = run_bass_kernel_spmd(nc, in_maps, list(range(N_CORES)))
    return assemble_output([res.results[i] for i in range(N_CORES)])
